# revision 70
# baseline (speedup 1.0000x reference)
"""Trainium2 Bass kernel for nn_BasePriorNetwork (4-layer dense transformer).

Sharding: data-parallel over batch (B=8) across 8 NeuronCores; weights
replicated (bf16). Activations feature-major [feat, token]; residual kept
f32. Key algebraic restructurings vs a straightforward lowering:
  - attn-LN is never materialized: l2norm makes Q/K invariant to the
    per-token rsqrt(var) scale, so Q/K/V consume the raw residual with a
    rank-1 mean-correction matmul; K's bias is pre-divided by the scale via
    a sqrt(var)-weighted rank-1 term; V is rescaled per token by r inside
    the PSUM->SBUF copy (Act scale).
  - softmax is left unnormalized: the post-Wo LayerNorm is invariant to a
    per-token positive scale, so the 1/sum(exp) divide cancels exactly.
  - rel-pos bias + causal mask are folded into a precomputed exp(bias)
    multiplier (exact zeros on masked entries).
  - rotary is applied as q*cosx + (P2@q)*sinx with a constant permutation
    matrix P2 on the PE, processing two heads per op.
  - all LN gains are folded into the adjacent weight matrices host-side.
"""
import sys, math
sys.path.insert(0, '/opt/trn_rl_repo')
import numpy as np
import ml_dtypes

import concourse.bass as bass
import concourse.bacc as bacc
import concourse.tile as tile
from concourse import mybir

f32 = mybir.dt.float32
f32r = mybir.dt.float32r
bf16 = mybir.dt.bfloat16
AF = mybir.ActivationFunctionType
ALU = mybir.AluOpType

B, N, D = 8, 515, 1024
H, DH, L = 8, 64, 4
FF = 4 * D
ROT = 32
NB, MAXD = 32, 128
EPS = 1e-5

NP = 520                      # padded tokens
QT = 260                      # query/free tile (2 per NP)
QTS = [(0, QT), (QT, QT)]
NK = 515                      # key row of the null key (= token rows + 1)
NMT = 8                       # feature tiles per 1024
TCH = [(0, 128), (128, 128), (256, 128), (384, 128), (512, 8)]
# chunks of key rows per qtile (causal-trimmed; chunk 4 holds the null key)
CHUNKS = [[0, 1, 2, 4], [0, 1, 2, 3, 4]]
NSLOT = 9
KW = [128, 128, 128, 128, 8]

# head-dim permutation: rows 0:16 even rot dims, 16:32 pass, 32:48 odd rot
# dims, 48:64 pass.
PERM = (list(range(0, ROT, 2)) + list(range(ROT, ROT + 16))
        + list(range(1, ROT, 2)) + list(range(ROT + 16, DH)))

BF = ml_dtypes.bfloat16


def _np_bias(emb):
    """bias[h, i, j'] as in reference (i query 0..N-1, j'=0 null, j'=t+1)."""
    q_pos = np.arange(N)
    k_pos = np.arange(N + 1)
    rel = k_pos[None, :] - q_pos[:, None]
    nn = np.maximum(-rel, 0)
    max_exact = NB // 2
    is_small = nn < max_exact
    nf = np.maximum(nn, 1).astype(np.float32)
    val_large = max_exact + (
        np.log(nf / np.float32(max_exact)).astype(np.float32)
        / np.float32(math.log(MAXD / max_exact)) * np.float32(NB - max_exact)
    ).astype(np.int32)
    val_large = np.minimum(val_large, NB - 1)
    bucket = np.where(is_small, nn, val_large)          # [N, N+1]
    return np.transpose(emb[bucket], (2, 0, 1)).astype(np.float32)  # [H,N,N+1]


def _np_ebias(emb):
    """ebias[h, keyrow, qcol] = exp(bias) with causal+pad mask folded (0).
    Key row t = token t; null key at row NK=515."""
    bias = _np_bias(emb)                                 # [H, N, N+1]
    eb = np.zeros((H, NP, NP), np.float32)
    t = np.arange(N)
    vals = np.exp(bias[:, :, 1:])                        # [H, i, t]
    allowed = (t[None, :] <= np.arange(N)[:, None])      # t <= i
    vals = vals * allowed[None, :, :]
    eb[:, :N, :N] = np.transpose(vals, (0, 2, 1))        # [H, keyrow, i]
    eb[:, NK, :N] = np.exp(bias[:, :, 0])                # null row per query i
    eb[:, NK, N:] = 1.0                                  # padded queries
    # slot layout [128, H, NSLOT, QT]
    out = np.zeros((128, H, NSLOT, QT), np.float32)
    for qi in range(2):
        for ci, c in enumerate(CHUNKS[qi]):
            s = ci if qi == 0 else 4 + ci
            kw = KW[c]
            out[:kw, :, s, :] = np.transpose(
                eb[:, 128 * c:128 * c + kw, qi * QT:(qi + 1) * QT], (1, 0, 2))
    return out.astype(BF)


def _host_prep(inputs):
    f = np.float32
    x = np.asarray(inputs['x'], f)
    Wq = np.asarray(inputs['Wq'], f)
    Wkv = np.asarray(inputs['Wkv'], f)
    bkv = np.asarray(inputs['bkv'], f)
    null_kv = np.asarray(inputs['null_kv'], f)
    Wo = np.asarray(inputs['Wo'], f)
    ga_all = np.asarray(inputs['attn_norm_g'], f)
    go_all = np.asarray(inputs['out_norm_g'], f)
    gf_all = np.asarray(inputs['ff_norm_g'], f)
    Wff1 = np.asarray(inputs['Wff1'], f)
    Wff2 = np.asarray(inputs['Wff2'], f)
    gfin = np.asarray(inputs['final_norm_g'], f)
    Wproj = np.asarray(inputs['Wproj'], f)

    d = {}
    xT = np.zeros((B, D, NP), f)
    xT[:, :, :N] = np.transpose(x, (0, 2, 1))
    d['xT'] = xT

    d['ebias'] = np.ascontiguousarray(
        _np_ebias(np.asarray(inputs['relpos_emb'], f)).reshape(128, H * NSLOT * QT))

    # rotary tables
    inv_freq = (1.0 / (10000.0 ** (np.arange(0, ROT, 2, dtype=f) / ROT))).astype(f)
    th = np.arange(NP, dtype=f)[None, :] * inv_freq[:, None]     # [16, NP]
    cos, sin = np.cos(th).astype(f), np.sin(th).astype(f)
    one16, zero16 = np.ones_like(cos), np.zeros_like(cos)
    cosx = np.concatenate([cos, one16, cos, one16], 0)           # [64, NP]
    sinx = np.concatenate([-sin, zero16, sin, zero16], 0)
    d['cosx'] = np.ascontiguousarray(np.concatenate([cosx, cosx], 0)).astype(BF)
    d['sinx'] = np.ascontiguousarray(np.concatenate([sinx, sinx], 0)).astype(BF)

    p2 = np.zeros((64, 64), f)
    for i in range(16):
        p2[32 + i, i] = 1.0      # out even slot <- in odd row
        p2[i, 32 + i] = 1.0      # out odd slot <- in even row
    d['p2k'] = p2
    p2q = np.zeros((128, 128), f)
    p2q[:64, :64] = p2
    p2q[64:, 64:] = p2
    d['p2q'] = p2q

    mh2 = np.zeros((128, 2), f)
    mh2[:64, 0] = 1.0
    mh2[64:, 1] = 1.0
    d['mh2'] = mh2
    bch2 = np.zeros((2, 128), f)
    bch2[0, :64] = 1.0
    bch2[1, 64:] = 1.0
    d['bch2'] = bch2
    d['mh64'] = np.ones((64, 1), f)
    d['ones1x64'] = np.ones((1, 64), f).astype(BF)
    d['ones1x128'] = np.ones((1, 128), f).astype(BF)
    d['onesD'] = np.full((128, 1), 1.0 / D, f)
    d['one1'] = np.ones((1, 2), f)
    d['ones_c'] = np.ones((128, 64), f)
    d['onesb'] = np.ones((128, 1), f).astype(BF)

    go_safe = np.where(np.abs(go_all) < 1e-12, 1.0, go_all)
    d['invgo'] = np.ascontiguousarray(
        (1.0 / go_safe).reshape(L * 8, 128).T.astype(f))          # [128, L*8] f32
    d['gorow'] = np.ascontiguousarray(go_all.reshape(1, L * D)).astype(BF)

    # per-layer folded weights
    wq = np.zeros((L, 4, 128, 8, 128), f)
    wq1n = np.zeros((L, 4, 128), f)
    wk = np.zeros((L, 128, 8, DH), f)
    wv = np.zeros((L, 128, 8, DH), f)
    kvec = np.zeros((L, 1, 4 * DH), f)   # wk1n | bkp | wv1n | bvv
    nullk = np.zeros((L, 128, 1), f)
    nullv = np.zeros((L, 1, DH), f)
    wo = np.zeros((L, 8, 64, 8, 128), f)
    wo1n = np.zeros((L, 64, 8), f)
    wff1 = np.zeros((L, 32, 2, 128, 8, 128), f)
    wff2 = np.zeros((L, 4, 8, 128, 8, 128), f)
    for l in range(L):
        ga, go, gf = ga_all[l], go_all[l], gf_all[l]
        Wq_g = ga[:, None] * Wq[l]                                # [D, 512]
        for p in range(4):
            pw = np.concatenate(
                [Wq_g[:, (2 * p) * DH:(2 * p + 1) * DH][:, PERM],
                 Wq_g[:, (2 * p + 1) * DH:(2 * p + 2) * DH][:, PERM]], 1)
            wq[l, p] = pw.reshape(8, 128, 128).transpose(1, 0, 2)
            wq1n[l, p] = -pw.sum(0)
        Wk_g = (ga[:, None] * Wkv[l][:, :DH])[:, PERM]
        wk[l] = Wk_g.reshape(8, 128, DH).transpose(1, 0, 2)
        Wv_g = ga[:, None] * Wkv[l][:, DH:]
        wv[l] = Wv_g.reshape(8, 128, DH).transpose(1, 0, 2)
        kvec[l, 0, 0:DH] = -Wk_g.sum(0)
        kvec[l, 0, DH:2 * DH] = bkv[l, :DH][PERM]
        kvec[l, 0, 2 * DH:3 * DH] = -Wv_g.sum(0)
        kvec[l, 0, 3 * DH:] = bkv[l, DH:]
        kn = null_kv[l, 0][PERM]
        kh = 4.0 * kn / max(np.linalg.norm(kn), 1e-12)
        nullk[l, :64, 0] = kh
        nullk[l, 64:, 0] = kh
        nullv[l, 0] = null_kv[l, 1]
        Wo_g = Wo[l] * go[None, :]                                # [512, D]
        wo[l] = Wo_g.reshape(8, 64, 8, 128).transpose(2, 1, 0, 3)
        wo1n[l] = -(Wo[l].sum(1) / D).reshape(8, 64).T
        Wf1_g = gf[:, None] * Wff1[l]                             # [D, 2FF]
        a = Wf1_g[:, :FF].reshape(8, 128, 32, 128)
        g = Wf1_g[:, FF:].reshape(8, 128, 32, 128)
        wff1[l, :, 0] = a.transpose(2, 1, 0, 3)
        wff1[l, :, 1] = g.transpose(2, 1, 0, 3)
        wff2[l] = Wff2[l].reshape(4, 8, 128, 8, 128).transpose(0, 3, 2, 1, 4)
    d['wq'] = wq
    d['wq1n'] = np.ascontiguousarray(wq1n.reshape(L, 1, 4 * 128))
    d['wk'] = wk
    d['wv'] = wv
    d['kvec'] = kvec
    d['nullk'] = nullk.astype(BF)
    d['nullv'] = nullv.astype(BF)
    d['wo'] = wo.astype(BF)
    d['wo1n'] = wo1n.astype(BF)
    d['wff1'] = wff1.astype(BF)
    d['wff2'] = wff2.astype(BF)
    Wp_g = gfin[:, None] * Wproj
    d['wproj'] = np.ascontiguousarray(
        Wp_g.reshape(8, 128, 2, 512).transpose(2, 0, 1, 3)).astype(BF)
    return d


def _build():
    nc = bacc.Bacc("TRN2", target_bir_lowering=False, debug=False, num_devices=8)

    def P(name, shape, dt=bf16):
        return nc.declare_dram_parameter(name, list(shape), dt, isOutput=False)

    xT_d = P('xT', [D, NP], f32)
    eb_d = P('ebias', [128, H * NSLOT * QT])
    cosx_d = P('cosx', [128, NP])
    sinx_d = P('sinx', [128, NP])
    p2q_d = P('p2q', [128, 128], f32)
    p2k_d = P('p2k', [64, 64], f32)
    mh2_d = P('mh2', [128, 2], f32)
    bch2_d = P('bch2', [2, 128], f32)
    mh64_d = P('mh64', [64, 1], f32)
    o1x64_d = P('ones1x64', [1, 64])
    o1x128_d = P('ones1x128', [1, 128])
    onesD_d = P('onesD', [128, 1], f32)
    one1_d = P('one1', [1, 2], f32)
    onesb_d = P('onesb', [128, 1])
    onesc_d = P('ones_c', [128, 64], f32)
    invgo_d = P('invgo', [128, L * 8], f32)
    gorow_d = P('gorow', [1, L * D])
    wq_d = P('wq', [L, 4, 128, 8, 128], f32)
    wq1n_d = P('wq1n', [L, 1, 4 * 128], f32)
    wk_d = P('wk', [L, 128, 8, DH], f32)
    wv_d = P('wv', [L, 128, 8, DH], f32)
    kvec_d = P('kvec', [L, 1, 4 * DH], f32)
    nullk_d = P('nullk', [L, 128, 1])
    nullv_d = P('nullv', [L, 1, DH])
    wo_d = P('wo', [L, 8, 64, 8, 128])
    wo1n_d = P('wo1n', [L, 64, 8])
    wff1_d = P('wff1', [L, 32, 2, 128, 8, 128])
    wff2_d = P('wff2', [L, 4, 8, 128, 8, 128])
    wproj_d = P('wproj', [2, 8, 128, 512])
    out_d = nc.declare_dram_parameter('out', [N, D], f32, isOutput=True)

    R = f32r

    from contextlib import ExitStack
    with ExitStack() as _es:
        _es.enter_context(
            nc.allow_low_precision("bf16 weights/activations; fp32 accumulation"))
        tc = _es.enter_context(tile.TileContext(nc))
        pool = lambda **kw: _es.enter_context(tc.tile_pool(**kw))
        cpool = pool(name="const", bufs=1)
        rpool = pool(name="res", bufs=1)
        wqp = pool(name="wq", bufs=2)
        wsp = pool(name="wsmall", bufs=2)
        wop = pool(name="wo", bufs=9)
        wf1p = pool(name="wff1", bufs=4)
        wf2p = pool(name="wff2", bufs=2)
        wpp = pool(name="wproj", bufs=8)
        scrp = pool(name="scr", bufs=3)
        rotp = pool(name="rot", bufs=3)
        attp = pool(name="attn", bufs=4)
        sgp = pool(name="sg", bufs=3)
        vecp = pool(name="vec", bufs=3)
        ps = pool(name="ps", bufs=4, space="PSUM")
        psacc = pool(name="psacc", bufs=2, space="PSUM")
        psbc = pool(name="psbc", bufs=2, space="PSUM")
        if True:

            # ---------- constants ----------
            cosx_t = cpool.tile([128, NP], bf16, tag="cosx")
            nc.scalar.dma_start(cosx_t[:], cosx_d[:])
            sinx_t = cpool.tile([128, NP], bf16, tag="sinx")
            nc.scalar.dma_start(sinx_t[:], sinx_d[:])
            p2q_t = cpool.tile([128, 128], R, tag="p2q")
            nc.scalar.dma_start(p2q_t[:], p2q_d[:].bitcast(R))
            p2k_t = cpool.tile([64, 64], R, tag="p2k")
            nc.scalar.dma_start(p2k_t[:], p2k_d[:].bitcast(R))
            mh2_t = cpool.tile([128, 2], R, tag="mh2")
            nc.scalar.dma_start(mh2_t[:], mh2_d[:].bitcast(R))
            bch2_t = cpool.tile([2, 128], R, tag="bch2")
            nc.scalar.dma_start(bch2_t[:], bch2_d[:].bitcast(R))
            mh64_t = cpool.tile([64, 1], R, tag="mh64")
            nc.scalar.dma_start(mh64_t[:], mh64_d[:].bitcast(R))
            onesD_t = cpool.tile([128, 1], R, tag="onesD")
            nc.scalar.dma_start(onesD_t[:], onesD_d[:].bitcast(R))
            one1_t = cpool.tile([1, 2], R, tag="one1")
            nc.scalar.dma_start(one1_t[:], one1_d[:].bitcast(R))
            onesc_t = cpool.tile([128, 64], R, tag="onesc")
            nc.scalar.dma_start(onesc_t[:], onesc_d[:].bitcast(R))
            invgo_t = cpool.tile([128, L * 8], f32, tag="invgo")
            nc.scalar.dma_start(invgo_t[:], invgo_d[:])

            epsc = cpool.tile([128, 1], f32, tag="epsc")
            nc.gpsimd.memset(epsc[:], EPS)
            eps12 = cpool.tile([128, 1], f32, tag="eps12")
            nc.gpsimd.memset(eps12[:], 1e-12)

            # ---------- persistent activations ----------
            xt = rpool.tile([128, NMT * NP], R, tag="x")
            for mt in range(NMT):
                nc.gpsimd.dma_start(
                    xt[:, mt * NP:(mt + 1) * NP],
                    xT_d[mt * 128:(mt + 1) * 128, :].bitcast(R))
            o2xn = rpool.tile([128, NMT * NP], bf16, tag="o2xn")
            qhat = rpool.tile([128, 4 * NP], R, tag="qhat")
            khat = rpool.tile([128, NP], R, tag="khat")
            vaug = rpool.tile([128, 5 * 65], bf16, tag="vaug")
            oT = rpool.tile([64, H * NP], bf16, tag="oT")
            sff = rpool.tile([128, NMT * NP], bf16, tag="sff")
            m_sb = rpool.tile([1, NP], R, tag="m_sb")
            s_sb = rpool.tile([1, NP], R, tag="s_sb")
            r_sb = rpool.tile([1, NP], R, tag="r_sb")
            mo_sb = rpool.tile([1, NP], bf16, tag="mo_sb")
            for c in range(5):
                nc.gpsimd.dma_start(vaug[:, c * 65 + 64:c * 65 + 65], onesb_d[:])
            # ebias is large (4.3MB) and first needed by layer-0 attention:
            # issue after xT/layer-0 weight DMAs so they aren't starved.
            eb_t = cpool.tile([128, H * NSLOT * QT], bf16, tag="eb")
            nc.scalar.dma_start(eb_t[:], eb_d[:])

            def rot_l2(pq, np_, p2t, mht, qo, qw, dst):
                """rotary + l2norm(*4). pq: [np_, QT] PSUM -> dst (tile, off)."""
                qs = rotp.tile([128, QT], R, tag="qs")
                nc.scalar.activation(qs[0:np_, :], pq[:, :], AF.Copy)
                rps = ps.tile([128, QT], f32, tag="mm")
                nc.tensor.matmul(rps[0:np_, :], p2t[:], qs[0:np_, :],
                                 start=True, stop=True)
                t1 = rotp.tile([128, QT], R, tag="t1")
                nc.vector.scalar_tensor_tensor(
                    t1[0:np_, :], qs[0:np_, :], 1.0, cosx_t[0:np_, qo:qo + qw],
                    ALU.mult, ALU.mult)
                t2 = rotp.tile([128, QT], R, tag="t2")
                nc.vector.scalar_tensor_tensor(
                    t2[0:np_, :], rps[0:np_, :], 1.0, sinx_t[0:np_, qo:qo + qw],
                    ALU.mult, ALU.mult)
                qr = rotp.tile([128, QT], R, tag="qr")
                nc.gpsimd.tensor_tensor(qr[0:np_, :], t1[0:np_, :], t2[0:np_, :],
                                        ALU.add)
                sq = rotp.tile([128, QT], R, tag="sq", bufs=2)
                nc.scalar.activation(sq[0:np_, :], qr[0:np_, :], AF.Square)
                nh = np_ // 64
                ssq = psbc.tile([1, QT] if nh == 1 else [2, QT], f32, tag="bc")
                nc.tensor.matmul(ssq[:], mht[:], sq[0:np_, :], start=True, stop=True)
                sh = vecp.tile([2, QT], f32, tag="sh")
                nc.scalar.activation(sh[0:nh, :], ssq[:], AF.Sqrt,
                                     scale=1.0 / 16.0, bias=eps12[0:nh, :])
                rh = vecp.tile([2, QT], bf16, tag="rh")
                nc.vector.reciprocal(rh[0:nh, :], sh[0:nh, :])
                dt_, off = dst
                if nh == 1:
                    bcq = rotp.tile([128, QT], R, tag="bcq", bufs=2)
                    nc.gpsimd.partition_broadcast(bcq[0:64, :].bitcast(f32), rh[0:1, :].bitcast(f32), 64)
                    nc.vector.scalar_tensor_tensor(
                        dt_[0:np_, off:off + qw], qr[0:np_, :], 1.0,
                        bcq[0:np_, :], ALU.mult, ALU.mult)
                else:
                    bcp = psbc.tile([128, QT], f32, tag="bc")
                    nc.tensor.matmul(bcp[:], bch2_t[:], rh[0:nh, :],
                                     start=True, stop=True)
                    nc.vector.scalar_tensor_tensor(
                        dt_[0:np_, off:off + qw], qr[0:np_, :], 1.0,
                        bcp[0:np_, :], ALU.mult, ALU.mult)

            # ================= layers =================
            for l in range(L):
                # ---- per-layer small weights ----
                wkt = wsp.tile([128, 8 * DH], R, tag="wk")
                nc.gpsimd.dma_start(
                    wkt[:].rearrange("p (c m) -> p c m", c=8), wk_d[l].bitcast(R))
                wvt = wsp.tile([128, 8 * DH], R, tag="wv")
                nc.gpsimd.dma_start(
                    wvt[:].rearrange("p (c m) -> p c m", c=8), wv_d[l].bitcast(R))
                kvec_t = wsp.tile([1, 4 * DH], R, tag="kvec")
                nc.gpsimd.dma_start(kvec_t[:], kvec_d[l].bitcast(R))
                wk1n_t = kvec_t[0:1, 0:DH]
                bkp_t = kvec_t[0:1, DH:2 * DH]
                wv1n_t = kvec_t[0:1, 2 * DH:3 * DH]
                bvv_t = kvec_t[0:1, 3 * DH:4 * DH]
                wq1n_t = wsp.tile([1, 4 * 128], R, tag="wq1n")
                nc.gpsimd.dma_start(wq1n_t[:], wq1n_d[l].bitcast(R))
                wo1n_t = wsp.tile([64, 8], bf16, tag="wo1n")
                nc.gpsimd.dma_start(wo1n_t[:], wo1n_d[l])
                gorow_t = wsp.tile([1, D], bf16, tag="gorow", bufs=1)
                nc.gpsimd.dma_start(gorow_t[:], gorow_d[0:1, l * D:(l + 1) * D])

                # ---- A: attn stats (m, s, r per token) ----
                for (qo, qw) in QTS:
                    s1 = psbc.tile([1, QT], f32, tag="bc")
                    s2 = psbc.tile([1, QT], f32, tag="bc")
                    for mt in range(NMT):
                        seg = xt[:, mt * NP + qo:mt * NP + qo + qw]
                        sq = scrp.tile([128, QT], R, tag="sq")
                        nc.scalar.activation(sq[:], seg, AF.Square)
                        nc.tensor.matmul(s1[:], onesD_t[:], seg,
                                         start=(mt == 0), stop=(mt == NMT - 1))
                        nc.tensor.matmul(s2[:], onesD_t[:], sq[:],
                                         start=(mt == 0), stop=(mt == NMT - 1))
                    nc.vector.tensor_copy(m_sb[0:1, qo:qo + qw], s1[:])
                    msq = vecp.tile([1, QT], f32, tag="msq")
                    nc.scalar.activation(msq[:], s1[:], AF.Square)
                    v_v = vecp.tile([1, QT], f32, tag="v")
                    nc.vector.scalar_tensor_tensor(v_v[:], s2[:], 1.0, msq[:],
                                                   ALU.mult, ALU.subtract)
                    nc.scalar.activation(s_sb[0:1, qo:qo + qw], v_v[:], AF.Sqrt,
                                         bias=epsc[0:1, :])
                    nc.vector.reciprocal(r_sb[0:1, qo:qo + qw],
                                         s_sb[0:1, qo:qo + qw])

                # ---- B/C/D: K, Q, V -- mains pipelined ahead of the
                # stats-dependent finishers (corrections + rotary/l2norm) ----
                def k_main(qi):
                    qo, qw = QTS[qi]
                    pk = ps.tile([64, QT], f32, tag="mm")
                    for c in range(8):
                        nc.tensor.matmul(pk[:], wkt[:, c * DH:(c + 1) * DH],
                                         xt[:, c * NP + qo:c * NP + qo + qw],
                                         start=(c == 0), stop=False)
                    return pk

                def k_fin(qi, pk):
                    qo, qw = QTS[qi]
                    nc.tensor.matmul(pk[:], wk1n_t, m_sb[0:1, qo:qo + qw],
                                     start=False, stop=False)
                    nc.tensor.matmul(pk[:], bkp_t, s_sb[0:1, qo:qo + qw],
                                     start=False, stop=True)
                    rot_l2(pk, 64, p2k_t, mh64_t, qo, qw, (khat, qo))

                def q_main(p, wqt, qi):
                    qo, qw = QTS[qi]
                    pq = ps.tile([128, QT], f32, tag="mm")
                    for c in range(8):
                        nc.tensor.matmul(pq[:], wqt[:, c * 128:(c + 1) * 128],
                                         xt[:, c * NP + qo:c * NP + qo + qw],
                                         start=(c == 0), stop=False)
                    return pq

                def q_fin(p, qi, pq):
                    qo, qw = QTS[qi]
                    nc.tensor.matmul(pq[:], wq1n_t[0:1, p * 128:(p + 1) * 128],
                                     m_sb[0:1, qo:qo + qw],
                                     start=False, stop=True)
                    rot_l2(pq, 128, p2q_t, mh2_t, qo, qw, (qhat, p * NP + qo))

                def v_chunk(t):
                    to, tw = TCH[t]
                    rtk = psbc.tile([128, 2], f32, tag="bc")
                    nc.tensor.matmul(rtk[0:tw, :], r_sb[0:1, to:to + tw],
                                     one1_t[:], start=True, stop=True)
                    pv = ps.tile([128, DH], f32, tag="mm")
                    for c in range(8):
                        nc.tensor.matmul(pv[0:tw, :],
                                         xt[:, c * NP + to:c * NP + to + tw],
                                         wvt[:, c * DH:(c + 1) * DH],
                                         start=(c == 0), stop=False)
                    nc.tensor.matmul(pv[0:tw, :], m_sb[0:1, to:to + tw],
                                     wv1n_t, start=False, stop=False)
                    nc.tensor.matmul(pv[0:tw, :], s_sb[0:1, to:to + tw],
                                     bvv_t, start=False, stop=True)
                    with tc.high_priority():
                        rts = vecp.tile([128, 1], f32, tag="rts")
                        nc.vector.tensor_copy(rts[0:tw, :], rtk[0:tw, 0:1])
                        nc.scalar.activation(vaug[0:tw, t * 65:t * 65 + DH],
                                             pv[0:tw, :], AF.Copy,
                                             scale=rts[0:tw, :])

                wqts = {}
                def load_wq(p):
                    wqt = wqp.tile([128, 8 * 128], R, tag="wq")
                    nc.sync.dma_start(
                        wqt[:].rearrange("p (c m) -> p c m", c=8),
                        wq_d[l, p].bitcast(R))
                    wqts[p] = wqt

                load_wq(0)
                pk0 = k_main(0)
                pk1 = k_main(1)
                load_wq(1)
                pq = {}
                pq[(0, 0)] = q_main(0, wqts[0], 0)
                k_fin(0, pk0)
                pq[(0, 1)] = q_main(0, wqts[0], 1)
                k_fin(1, pk1)
                nc.gpsimd.dma_start(khat[64:128, 0:NP], khat[0:64, 0:NP])
                nc.gpsimd.dma_start(khat[:, NK:NK + 1], nullk_d[l])
                load_wq(2)
                pq[(1, 0)] = q_main(1, wqts[1], 0)
                q_fin(0, 0, pq[(0, 0)])
                pq[(1, 1)] = q_main(1, wqts[1], 1)
                q_fin(0, 1, pq[(0, 1)])
                load_wq(3)
                pq[(2, 0)] = q_main(2, wqts[2], 0)
                q_fin(1, 0, pq[(1, 0)])
                pq[(2, 1)] = q_main(2, wqts[2], 1)
                q_fin(1, 1, pq[(1, 1)])
                pq[(3, 0)] = q_main(3, wqts[3], 0)
                q_fin(2, 0, pq[(2, 0)])
                pq[(3, 1)] = q_main(3, wqts[3], 1)
                q_fin(2, 1, pq[(2, 1)])
                v_chunk(0)
                v_chunk(1)
                v_chunk(2)
                q_fin(3, 0, pq[(3, 0)])
                v_chunk(3)
                v_chunk(4)
                q_fin(3, 1, pq[(3, 1)])
                # null v at key row NK=515 -> chunk 4, partition 3
                nc.gpsimd.dma_start(vaug[3:4, 4 * 65:4 * 65 + DH], nullv_d[l])

                # ---- E: attention (unnormalized softmax) ----
                for h in range(H):
                    hb = 64 * (h & 1)
                    qcol = (h >> 1) * NP
                    for qi, (qo, qw) in enumerate(QTS):
                        av = psacc.tile([128, QT], f32, tag="av")
                        chunks = CHUNKS[qi]
                        for ci, c in enumerate(chunks):
                            kw = KW[c]
                            slot = (ci if qi == 0 else 4 + ci)
                            sp = ps.tile([128, QT], f32, tag="mm")
                            nc.tensor.matmul(
                                sp[0:kw, :],
                                khat[hb:hb + 64, 128 * c:128 * c + kw],
                                qhat[hb:hb + 64, qcol + qo:qcol + qo + qw],
                                start=True, stop=True)
                            aex = attp.tile([128, QT], bf16, tag="aex")
                            nc.scalar.activation(aex[0:kw, :], sp[0:kw, :], AF.Exp)
                            au = attp.tile([128, QT], bf16, tag="au")
                            ebs = (h * NSLOT + slot) * QT
                            with tc.high_priority():
                                nc.vector.scalar_tensor_tensor(
                                    au[0:kw, :], aex[0:kw, :], 1.0,
                                    eb_t[0:kw, ebs:ebs + QT], ALU.mult, ALU.mult)
                            nc.tensor.matmul(av[0:65, :],
                                             vaug[0:kw, c * 65:(c + 1) * 65],
                                             au[0:kw, :], start=(ci == 0),
                                             stop=(ci == len(chunks) - 1))
                        rd = scrp.tile([128, QT], R, tag="rd", bufs=2)
                        nc.vector.reciprocal(rd[64:65, :], av[64:65, :])
                        bco = psbc.tile([64, QT], f32, tag="bc")
                        nc.tensor.matmul(bco[0:64, :], onesc_t[64:65, :],
                                         rd[64:65, :], start=True, stop=True)
                        bcos = attp.tile([64, QT], bf16, tag="bcos", bufs=2)
                        nc.vector.tensor_copy(bcos[:], bco[0:64, :])
                        nc.vector.scalar_tensor_tensor(
                            oT[0:64, h * NP + qo:h * NP + qo + qw],
                            av[0:64, :], 1.0, bcos[:], ALU.mult, ALU.mult)

                # ---- F: Wo + out-LN + residual ----
                wots = []
                for mt in range(NMT):
                    wot = wop.tile([64, 8 * 128], bf16, tag="wo")
                    nc.gpsimd.dma_start(
                        wot[:].rearrange("p (c m) -> p c m", c=8), wo_d[l, mt])
                    wots.append(wot)
                for (qo, qw) in QTS:
                    mo = psbc.tile([1, QT], f32, tag="bc")
                    for c in range(8):
                        nc.tensor.matmul(mo[:], wo1n_t[:, c:c + 1],
                                         oT[0:64, c * NP + qo:c * NP + qo + qw],
                                         start=(c == 0), stop=(c == 7))
                    nc.vector.tensor_copy(mo_sb[0:1, qo:qo + qw], mo[:])
                    s2c = psbc.tile([1, QT], f32, tag="bc")
                    for mt in range(NMT):
                        pl = ps.tile([128, QT], f32, tag="mm")
                        for c in range(8):
                            nc.tensor.matmul(pl[:],
                                             wots[mt][:, c * 128:(c + 1) * 128],
                                             oT[0:64, c * NP + qo:c * NP + qo + qw],
                                             start=(c == 0), stop=False)
                        nc.tensor.matmul(pl[:],
                                         gorow_t[0:1, mt * 128:(mt + 1) * 128],
                                         mo_sb[0:1, qo:qo + qw],
                                         start=False, stop=True)
                        sqo = scrp.tile([128, QT], R, tag="sq")
                        nc.scalar.activation(sqo[:], pl[:], AF.Square,
                                             scale=invgo_t[:, l * 8 + mt:
                                                           l * 8 + mt + 1])
                        nc.tensor.matmul(s2c[:], onesD_t[:], sqo[:],
                                         start=(mt == 0), stop=(mt == NMT - 1))
                        nc.scalar.activation(
                            o2xn[:, mt * NP + qo:mt * NP + qo + qw],
                            pl[:], AF.Copy)
                    so = vecp.tile([1, QT], f32, tag="so")
                    nc.scalar.activation(so[:], s2c[:], AF.Sqrt, bias=epsc[0:1, :])
                    ro = vecp.tile([1, QT], bf16, tag="ro")
                    nc.vector.reciprocal(ro[:], so[:])
                    rb = sgp.tile([128, QT], f32, tag="rbb", bufs=2)
                    nc.gpsimd.partition_broadcast(rb[:], ro[:], 128)
                    for mt in range(NMT):
                        eng = nc.vector if mt % 2 == 0 else nc.gpsimd
                        tt = scrp.tile([128, QT], bf16, tag="tt")
                        eng.tensor_tensor(
                            tt[:], o2xn[:, mt * NP + qo:mt * NP + qo + qw],
                            rb[:], ALU.mult)
                        xcols = xt[:, mt * NP + qo:mt * NP + qo + qw]
                        eng.tensor_tensor(xcols, xcols, tt[:], ALU.add)

                # ---- G: ff-LN -> xn (per qtile, interleaved with FFN
                # block 0 so PE keeps working during the q1 stats chain) ----
                def ff_ln(qi):
                    qo, qw = QTS[qi]
                    s1 = psbc.tile([1, QT], f32, tag="bc")
                    s2 = psbc.tile([1, QT], f32, tag="bc")
                    for mt in range(NMT):
                        seg = xt[:, mt * NP + qo:mt * NP + qo + qw]
                        sq = scrp.tile([128, QT], R, tag="sq")
                        nc.scalar.activation(sq[:], seg, AF.Square)
                        nc.tensor.matmul(s1[:], onesD_t[:], seg,
                                         start=(mt == 0), stop=(mt == NMT - 1))
                        nc.tensor.matmul(s2[:], onesD_t[:], sq[:],
                                         start=(mt == 0), stop=(mt == NMT - 1))
                    msq = vecp.tile([1, QT], f32, tag="vf")
                    nc.scalar.activation(msq[:], s1[:], AF.Square)
                    v_v = vecp.tile([1, QT], f32, tag="vf")
                    nc.vector.scalar_tensor_tensor(v_v[:], s2[:], 1.0, msq[:],
                                                   ALU.mult, ALU.subtract)
                    sf = vecp.tile([1, QT], f32, tag="vf")
                    nc.scalar.activation(sf[:], v_v[:], AF.Sqrt, bias=epsc[0:1, :])
                    rf = vecp.tile([1, QT], f32, tag="vbr")
                    nc.vector.reciprocal(rf[:], sf[:])
                    mrv = vecp.tile([1, QT], f32, tag="vbr")
                    nc.vector.scalar_tensor_tensor(mrv[:], s1[:], 1.0, rf[:],
                                                   ALU.mult, ALU.mult)
                    rb = sgp.tile([128, QT], f32, tag="rbb", bufs=2)
                    nc.gpsimd.partition_broadcast(rb[:], rf[:], 128)
                    mrb = sgp.tile([128, QT], f32, tag="mrbb", bufs=2)
                    nc.gpsimd.partition_broadcast(mrb[:], mrv[:], 128)
                    for mt in range(NMT):
                        eng = nc.vector if mt % 2 == 0 else nc.gpsimd
                        tt = scrp.tile([128, QT], bf16, tag="tt")
                        eng.tensor_tensor(
                            tt[:], xt[:, mt * NP + qo:mt * NP + qo + qw],
                            rb[:], ALU.mult)
                        eng.tensor_tensor(
                            o2xn[:, mt * NP + qo:mt * NP + qo + qw],
                            tt[:], mrb[:], ALU.subtract)

                # ---- H: FFN (SwiGLU), 4 halves of 8 s-blocks ----
                def ffn1_qt(mi, wga, wgg, qi):
                    qo, qw = QTS[qi]
                    pg = ps.tile([128, QT], f32, tag="mm")
                    for c in range(8):
                        nc.tensor.matmul(pg[:], wgg[:, c * 128:(c + 1) * 128],
                                         o2xn[:, c * NP + qo:c * NP + qo + qw],
                                         start=(c == 0), stop=(c == 7))
                    sig = sgp.tile([128, QT], bf16, tag="sig")
                    nc.scalar.activation(sig[:], pg[:], AF.Sigmoid)
                    pa = ps.tile([128, QT], f32, tag="mm")
                    for c in range(8):
                        nc.tensor.matmul(pa[:], wga[:, c * 128:(c + 1) * 128],
                                         o2xn[:, c * NP + qo:c * NP + qo + qw],
                                         start=(c == 0), stop=(c == 7))
                    gs = sgp.tile([128, QT], bf16, tag="ag")
                    nc.vector.scalar_tensor_tensor(
                        gs[:], pg[:], 1.0, sig[:], ALU.mult, ALU.mult)
                    nc.vector.scalar_tensor_tensor(
                        sff[:, mi * NP + qo:mi * NP + qo + qw],
                        pa[:], 1.0, gs[:], ALU.mult, ALU.mult)

                def load_wff1(m):
                    wga = wf1p.tile([128, 8 * 128], bf16, tag="wff1")
                    nc.sync.dma_start(
                        wga[:].rearrange("p (c m) -> p c m", c=8),
                        wff1_d[l, m, 0])
                    wgg = wf1p.tile([128, 8 * 128], bf16, tag="wff1")
                    nc.sync.dma_start(
                        wgg[:].rearrange("p (c m) -> p c m", c=8),
                        wff1_d[l, m, 1])
                    return wga, wgg

                ff_ln(0)
                w00 = load_wff1(0)
                ffn1_qt(0, w00[0], w00[1], 0)
                ff_ln(1)
                ffn1_qt(0, w00[0], w00[1], 1)
                for half in range(4):
                    for mi in range(8):
                        if half == 0 and mi == 0:
                            continue
                        m = half * 8 + mi
                        wga, wgg = load_wff1(m)
                        for qi in range(2):
                            ffn1_qt(mi, wga, wgg, qi)
                    for mt in range(NMT):
                        w2 = wf2p.tile([128, 8 * 128], bf16, tag="wff2")
                        nc.sync.dma_start(
                            w2[:].rearrange("p (c m) -> p c m", c=8),
                            wff2_d[l, half, mt])
                        for (qo, qw) in QTS:
                            pl = ps.tile([128, QT], f32, tag="mm")
                            for c in range(8):
                                nc.tensor.matmul(
                                    pl[:], w2[:, c * 128:(c + 1) * 128],
                                    sff[:, c * NP + qo:c * NP + qo + qw],
                                    start=(c == 0), stop=(c == 7))
                            xcols = xt[:, mt * NP + qo:mt * NP + qo + qw]
                            nc.gpsimd.tensor_tensor(xcols, xcols, pl[:], ALU.add)

            # ================= final stable LN + Wproj =================
            xm = sff[:, 0:NP]
            nc.vector.tensor_tensor(xm, xt[:, 0:NP], xt[:, NP:2 * NP], ALU.max)
            for mt in range(2, NMT):
                nc.vector.tensor_tensor(xm, xm, xt[:, mt * NP:(mt + 1) * NP],
                                        ALU.max)
            mxb = sff[:, NP:2 * NP]
            from concourse import bass_isa
            nc.gpsimd.partition_all_reduce(mxb, xm, 128, bass_isa.ReduceOp.max)

            for (qo, qw) in QTS:
                s1 = psbc.tile([1, QT], f32, tag="bc")
                s2 = psbc.tile([1, QT], f32, tag="bc")
                for mt in range(NMT):
                    seg = xt[:, mt * NP + qo:mt * NP + qo + qw]
                    sq = scrp.tile([128, QT], R, tag="sq")
                    nc.scalar.activation(sq[:], seg, AF.Square)
                    nc.tensor.matmul(s1[:], onesD_t[:], seg,
                                     start=(mt == 0), stop=(mt == NMT - 1))
                    nc.tensor.matmul(s2[:], onesD_t[:], sq[:],
                                     start=(mt == 0), stop=(mt == NMT - 1))
                msq = vecp.tile([1, QT], f32, tag="msq")
                nc.scalar.activation(msq[:], s1[:], AF.Square)
                v_v = vecp.tile([1, QT], f32, tag="v")
                nc.vector.scalar_tensor_tensor(v_v[:], s2[:], 1.0, msq[:],
                                               ALU.mult, ALU.subtract)
                mxsq = vecp.tile([1, QT], f32, tag="mxsq")
                nc.scalar.activation(mxsq[:], sff[0:1, NP + qo:NP + qo + qw], AF.Square)
                veps = vecp.tile([1, QT], f32, tag="veps")
                nc.vector.scalar_tensor_tensor(veps[:], mxsq[:], EPS, v_v[:],
                                               ALU.mult, ALU.add)
                sf = vecp.tile([1, QT], f32, tag="sf")
                nc.scalar.activation(sf[:], veps[:], AF.Sqrt)
                rf = vecp.tile([1, QT], bf16, tag="rf")
                nc.vector.reciprocal(rf[:], sf[:])
                mrv = vecp.tile([1, QT], bf16, tag="mrv")
                nc.vector.scalar_tensor_tensor(mrv[:], s1[:], 1.0, rf[:],
                                               ALU.mult, ALU.mult)
                rb = sgp.tile([128, QT], bf16, tag="rbb")
                nc.gpsimd.partition_broadcast(rb[:], rf[:], 128)
                mrb = sgp.tile([128, QT], bf16, tag="mrbb")
                nc.gpsimd.partition_broadcast(mrb[:], mrv[:], 128)
                for mt in range(NMT):
                    tt = scrp.tile([128, QT], bf16, tag="tt")
                    nc.vector.scalar_tensor_tensor(
                        tt[:], xt[:, mt * NP + qo:mt * NP + qo + qw], 1.0,
                        rb[:], ALU.mult, ALU.mult)
                    nc.vector.scalar_tensor_tensor(
                        o2xn[:, mt * NP + qo:mt * NP + qo + qw],
                        tt[:], 1.0, mrb[:], ALU.mult, ALU.subtract)

            for half in range(2):
                wps = []
                for c in range(8):
                    wp = wpp.tile([128, 512], bf16, tag=f"wp{c}")
                    nc.sync.dma_start(wp[:], wproj_d[half, c])
                    wps.append(wp)
                for t, (to, tw) in enumerate(TCH):
                    rtw = min(tw, max(0, N - to))
                    if rtw == 0:
                        continue
                    pn = psacc.tile([128, 512], f32, tag="av")
                    for c in range(8):
                        nc.tensor.matmul(pn[0:tw, :],
                                         o2xn[:, c * NP + to:c * NP + to + tw],
                                         wps[c][:], start=(c == 0), stop=(c == 7))
                    st = scrp.tile([128, 512], f32, tag="outst", bufs=1)
                    nc.scalar.activation(st[0:rtw, :], pn[0:rtw, :], AF.Copy)
                    nc.sync.dma_start(out_d[to:to + rtw, half * 512:(half + 1) * 512],
                                      st[0:rtw, :])

    nc.compile()
    return nc


_CACHE = {}


def _get_program():
    if 'nc' not in _CACHE:
        _CACHE['nc'] = _build()
    return _CACHE['nc']


def kernel(**inputs) -> np.ndarray:
    from concourse.bass_utils import run_bass_kernel_spmd
    host = _host_prep(inputs)
    nc = _get_program()
    shared = {k: v for k, v in host.items() if k != 'xT'}
    in_maps = [dict(shared, xT=np.ascontiguousarray(host['xT'][b])) for b in range(B)]
    res = run_bass_kernel_spmd(nc, in_maps, list(range(B)))
    out = np.stack([res.results[b]['out'] for b in range(B)], axis=0)
    _CACHE['last_results'] = res
    return out


# revision 71
# speedup vs baseline: 1.0104x; 1.0104x over previous
"""Trainium2 Bass kernel for nn_BasePriorNetwork (4-layer dense transformer).

Sharding: data-parallel over batch (B=8) across 8 NeuronCores; weights
replicated (bf16). Activations feature-major [feat, token]; residual kept
f32. Key algebraic restructurings vs a straightforward lowering:
  - attn-LN is never materialized: l2norm makes Q/K invariant to the
    per-token rsqrt(var) scale, so Q/K/V consume the raw residual with a
    rank-1 mean-correction matmul; K's bias is pre-divided by the scale via
    a sqrt(var)-weighted rank-1 term; V is rescaled per token by r inside
    the PSUM->SBUF copy (Act scale).
  - softmax is left unnormalized: the post-Wo LayerNorm is invariant to a
    per-token positive scale, so the 1/sum(exp) divide cancels exactly.
  - rel-pos bias + causal mask are folded into a precomputed exp(bias)
    multiplier (exact zeros on masked entries).
  - rotary is applied as q*cosx + (P2@q)*sinx with a constant permutation
    matrix P2 on the PE, processing two heads per op.
  - all LN gains are folded into the adjacent weight matrices host-side.
"""
import sys, math
sys.path.insert(0, '/opt/trn_rl_repo')
import numpy as np
import ml_dtypes

import concourse.bass as bass
import concourse.bacc as bacc
import concourse.tile as tile
from concourse import mybir

f32 = mybir.dt.float32
f32r = mybir.dt.float32r
bf16 = mybir.dt.bfloat16
AF = mybir.ActivationFunctionType
ALU = mybir.AluOpType

B, N, D = 8, 515, 1024
H, DH, L = 8, 64, 4
FF = 4 * D
ROT = 32
NB, MAXD = 32, 128
EPS = 1e-5

NP = 520                      # padded tokens
QT = 260                      # query/free tile (2 per NP)
QTS = [(0, QT), (QT, QT)]
NK = 515                      # key row of the null key (= token rows + 1)
NMT = 8                       # feature tiles per 1024
TCH = [(0, 128), (128, 128), (256, 128), (384, 128), (512, 8)]
# chunks of key rows per qtile (causal-trimmed; chunk 4 holds the null key)
CHUNKS = [[0, 1, 2, 4], [0, 1, 2, 3, 4]]
NSLOT = 9
KW = [128, 128, 128, 128, 8]

# head-dim permutation: rows 0:16 even rot dims, 16:32 pass, 32:48 odd rot
# dims, 48:64 pass.
PERM = (list(range(0, ROT, 2)) + list(range(ROT, ROT + 16))
        + list(range(1, ROT, 2)) + list(range(ROT + 16, DH)))

BF = ml_dtypes.bfloat16


def _np_bias(emb):
    """bias[h, i, j'] as in reference (i query 0..N-1, j'=0 null, j'=t+1)."""
    q_pos = np.arange(N)
    k_pos = np.arange(N + 1)
    rel = k_pos[None, :] - q_pos[:, None]
    nn = np.maximum(-rel, 0)
    max_exact = NB // 2
    is_small = nn < max_exact
    nf = np.maximum(nn, 1).astype(np.float32)
    val_large = max_exact + (
        np.log(nf / np.float32(max_exact)).astype(np.float32)
        / np.float32(math.log(MAXD / max_exact)) * np.float32(NB - max_exact)
    ).astype(np.int32)
    val_large = np.minimum(val_large, NB - 1)
    bucket = np.where(is_small, nn, val_large)          # [N, N+1]
    return np.transpose(emb[bucket], (2, 0, 1)).astype(np.float32)  # [H,N,N+1]


def _np_ebias(emb):
    """ebias[h, keyrow, qcol] = exp(bias) with causal+pad mask folded (0).
    Key row t = token t; null key at row NK=515."""
    bias = _np_bias(emb)                                 # [H, N, N+1]
    eb = np.zeros((H, NP, NP), np.float32)
    t = np.arange(N)
    vals = np.exp(bias[:, :, 1:])                        # [H, i, t]
    allowed = (t[None, :] <= np.arange(N)[:, None])      # t <= i
    vals = vals * allowed[None, :, :]
    eb[:, :N, :N] = np.transpose(vals, (0, 2, 1))        # [H, keyrow, i]
    eb[:, NK, :N] = np.exp(bias[:, :, 0])                # null row per query i
    eb[:, NK, N:] = 1.0                                  # padded queries
    # slot layout [128, H, NSLOT, QT]
    out = np.zeros((128, H, NSLOT, QT), np.float32)
    for qi in range(2):
        for ci, c in enumerate(CHUNKS[qi]):
            s = ci if qi == 0 else 4 + ci
            kw = KW[c]
            out[:kw, :, s, :] = np.transpose(
                eb[:, 128 * c:128 * c + kw, qi * QT:(qi + 1) * QT], (1, 0, 2))
    return out.astype(BF)


def _host_prep(inputs):
    f = np.float32
    x = np.asarray(inputs['x'], f)
    Wq = np.asarray(inputs['Wq'], f)
    Wkv = np.asarray(inputs['Wkv'], f)
    bkv = np.asarray(inputs['bkv'], f)
    null_kv = np.asarray(inputs['null_kv'], f)
    Wo = np.asarray(inputs['Wo'], f)
    ga_all = np.asarray(inputs['attn_norm_g'], f)
    go_all = np.asarray(inputs['out_norm_g'], f)
    gf_all = np.asarray(inputs['ff_norm_g'], f)
    Wff1 = np.asarray(inputs['Wff1'], f)
    Wff2 = np.asarray(inputs['Wff2'], f)
    gfin = np.asarray(inputs['final_norm_g'], f)
    Wproj = np.asarray(inputs['Wproj'], f)

    d = {}
    xT = np.zeros((B, D, NP), f)
    xT[:, :, :N] = np.transpose(x, (0, 2, 1))
    d['xT'] = xT

    d['ebias'] = np.ascontiguousarray(
        _np_ebias(np.asarray(inputs['relpos_emb'], f)).reshape(128, H * NSLOT * QT))

    # rotary tables
    inv_freq = (1.0 / (10000.0 ** (np.arange(0, ROT, 2, dtype=f) / ROT))).astype(f)
    th = np.arange(NP, dtype=f)[None, :] * inv_freq[:, None]     # [16, NP]
    cos, sin = np.cos(th).astype(f), np.sin(th).astype(f)
    one16, zero16 = np.ones_like(cos), np.zeros_like(cos)
    cosx = np.concatenate([cos, one16, cos, one16], 0)           # [64, NP]
    sinx = np.concatenate([-sin, zero16, sin, zero16], 0)
    d['cosx'] = np.ascontiguousarray(np.concatenate([cosx, cosx], 0)).astype(BF)
    d['sinx'] = np.ascontiguousarray(np.concatenate([sinx, sinx], 0)).astype(BF)

    p2 = np.zeros((64, 64), f)
    for i in range(16):
        p2[32 + i, i] = 1.0      # out even slot <- in odd row
        p2[i, 32 + i] = 1.0      # out odd slot <- in even row
    d['p2k'] = p2
    p2q = np.zeros((128, 128), f)
    p2q[:64, :64] = p2
    p2q[64:, 64:] = p2
    d['p2q'] = p2q

    mh2 = np.zeros((128, 2), f)
    mh2[:64, 0] = 1.0
    mh2[64:, 1] = 1.0
    d['mh2'] = mh2
    bch2 = np.zeros((2, 128), f)
    bch2[0, :64] = 1.0
    bch2[1, 64:] = 1.0
    d['bch2'] = bch2
    d['mh64'] = np.ones((64, 1), f)
    d['ones1x64'] = np.ones((1, 64), f).astype(BF)
    d['ones1x128'] = np.ones((1, 128), f).astype(BF)
    d['onesD'] = np.full((128, 1), 1.0 / D, f)
    d['one1'] = np.ones((1, 2), f)
    d['ones_c'] = np.ones((128, 64), f)
    d['onesb'] = np.ones((128, 1), f).astype(BF)

    go_safe = np.where(np.abs(go_all) < 1e-12, 1.0, go_all)
    d['invgo'] = np.ascontiguousarray(
        (1.0 / go_safe).reshape(L * 8, 128).T.astype(f))          # [128, L*8] f32
    d['gorow'] = np.ascontiguousarray(go_all.reshape(1, L * D)).astype(BF)

    # per-layer folded weights
    wq = np.zeros((L, 4, 128, 8, 128), f)
    wq1n = np.zeros((L, 4, 128), f)
    wk = np.zeros((L, 128, 8, DH), f)
    wv = np.zeros((L, 128, 8, DH), f)
    kvec = np.zeros((L, 1, 4 * DH), f)   # wk1n | bkp | wv1n | bvv
    nullk = np.zeros((L, 128, 1), f)
    nullv = np.zeros((L, 1, DH), f)
    wo = np.zeros((L, 8, 64, 8, 128), f)
    wo1n = np.zeros((L, 64, 8), f)
    wff1 = np.zeros((L, 32, 2, 128, 8, 128), f)
    wff2 = np.zeros((L, 4, 8, 128, 8, 128), f)
    for l in range(L):
        ga, go, gf = ga_all[l], go_all[l], gf_all[l]
        Wq_g = ga[:, None] * Wq[l]                                # [D, 512]
        for p in range(4):
            pw = np.concatenate(
                [Wq_g[:, (2 * p) * DH:(2 * p + 1) * DH][:, PERM],
                 Wq_g[:, (2 * p + 1) * DH:(2 * p + 2) * DH][:, PERM]], 1)
            wq[l, p] = pw.reshape(8, 128, 128).transpose(1, 0, 2)
            wq1n[l, p] = -pw.sum(0)
        Wk_g = (ga[:, None] * Wkv[l][:, :DH])[:, PERM]
        wk[l] = Wk_g.reshape(8, 128, DH).transpose(1, 0, 2)
        Wv_g = ga[:, None] * Wkv[l][:, DH:]
        wv[l] = Wv_g.reshape(8, 128, DH).transpose(1, 0, 2)
        kvec[l, 0, 0:DH] = -Wk_g.sum(0)
        kvec[l, 0, DH:2 * DH] = bkv[l, :DH][PERM]
        kvec[l, 0, 2 * DH:3 * DH] = -Wv_g.sum(0)
        kvec[l, 0, 3 * DH:] = bkv[l, DH:]
        kn = null_kv[l, 0][PERM]
        kh = 4.0 * kn / max(np.linalg.norm(kn), 1e-12)
        nullk[l, :64, 0] = kh
        nullk[l, 64:, 0] = kh
        nullv[l, 0] = null_kv[l, 1]
        Wo_g = Wo[l] * go[None, :]                                # [512, D]
        wo[l] = Wo_g.reshape(8, 64, 8, 128).transpose(2, 1, 0, 3)
        wo1n[l] = -(Wo[l].sum(1) / D).reshape(8, 64).T
        Wf1_g = gf[:, None] * Wff1[l]                             # [D, 2FF]
        a = Wf1_g[:, :FF].reshape(8, 128, 32, 128)
        g = Wf1_g[:, FF:].reshape(8, 128, 32, 128)
        wff1[l, :, 0] = a.transpose(2, 1, 0, 3)
        wff1[l, :, 1] = g.transpose(2, 1, 0, 3)
        wff2[l] = Wff2[l].reshape(4, 8, 128, 8, 128).transpose(0, 3, 2, 1, 4)
    d['wq'] = wq
    d['wq1n'] = np.ascontiguousarray(wq1n.reshape(L, 1, 4 * 128))
    d['wk'] = wk
    d['wv'] = wv
    d['kvec'] = kvec
    d['nullk'] = nullk.astype(BF)
    d['nullv'] = nullv.astype(BF)
    d['wo'] = wo.astype(BF)
    d['wo1n'] = wo1n.astype(BF)
    d['wff1'] = wff1.astype(BF)
    d['wff2'] = wff2.astype(BF)
    Wp_g = gfin[:, None] * Wproj
    d['wproj'] = np.ascontiguousarray(
        Wp_g.reshape(8, 128, 2, 512).transpose(2, 0, 1, 3)).astype(BF)
    return d


def _build():
    nc = bacc.Bacc("TRN2", target_bir_lowering=False, debug=False, num_devices=8)

    def P(name, shape, dt=bf16):
        return nc.declare_dram_parameter(name, list(shape), dt, isOutput=False)

    xT_d = P('xT', [D, NP], f32)
    eb_d = P('ebias', [128, H * NSLOT * QT])
    cosx_d = P('cosx', [128, NP])
    sinx_d = P('sinx', [128, NP])
    p2q_d = P('p2q', [128, 128], f32)
    p2k_d = P('p2k', [64, 64], f32)
    mh2_d = P('mh2', [128, 2], f32)
    bch2_d = P('bch2', [2, 128], f32)
    mh64_d = P('mh64', [64, 1], f32)
    o1x64_d = P('ones1x64', [1, 64])
    o1x128_d = P('ones1x128', [1, 128])
    onesD_d = P('onesD', [128, 1], f32)
    one1_d = P('one1', [1, 2], f32)
    onesb_d = P('onesb', [128, 1])
    onesc_d = P('ones_c', [128, 64], f32)
    invgo_d = P('invgo', [128, L * 8], f32)
    gorow_d = P('gorow', [1, L * D])
    wq_d = P('wq', [L, 4, 128, 8, 128], f32)
    wq1n_d = P('wq1n', [L, 1, 4 * 128], f32)
    wk_d = P('wk', [L, 128, 8, DH], f32)
    wv_d = P('wv', [L, 128, 8, DH], f32)
    kvec_d = P('kvec', [L, 1, 4 * DH], f32)
    nullk_d = P('nullk', [L, 128, 1])
    nullv_d = P('nullv', [L, 1, DH])
    wo_d = P('wo', [L, 8, 64, 8, 128])
    wo1n_d = P('wo1n', [L, 64, 8])
    wff1_d = P('wff1', [L, 32, 2, 128, 8, 128])
    wff2_d = P('wff2', [L, 4, 8, 128, 8, 128])
    wproj_d = P('wproj', [2, 8, 128, 512])
    out_d = nc.declare_dram_parameter('out', [N, D], f32, isOutput=True)

    R = f32r

    from contextlib import ExitStack
    with ExitStack() as _es:
        _es.enter_context(
            nc.allow_low_precision("bf16 weights/activations; fp32 accumulation"))
        tc = _es.enter_context(tile.TileContext(nc))
        pool = lambda **kw: _es.enter_context(tc.tile_pool(**kw))
        cpool = pool(name="const", bufs=1)
        rpool = pool(name="res", bufs=1)
        wqp = pool(name="wq", bufs=2)
        wsp = pool(name="wsmall", bufs=2)
        wop = pool(name="wo", bufs=9)
        wf1p = pool(name="wff1", bufs=5)
        wf2p = pool(name="wff2", bufs=2)
        wpp = pool(name="wproj", bufs=8)
        scrp = pool(name="scr", bufs=3)
        rotp = pool(name="rot", bufs=2)
        attp = pool(name="attn", bufs=3)
        sgp = pool(name="sg", bufs=3)
        vecp = pool(name="vec", bufs=3)
        ps = pool(name="ps", bufs=4, space="PSUM")
        psacc = pool(name="psacc", bufs=2, space="PSUM")
        psbc = pool(name="psbc", bufs=2, space="PSUM")
        if True:

            # ---------- constants ----------
            cosx_t = cpool.tile([128, NP], bf16, tag="cosx")
            nc.scalar.dma_start(cosx_t[:], cosx_d[:])
            sinx_t = cpool.tile([128, NP], bf16, tag="sinx")
            nc.scalar.dma_start(sinx_t[:], sinx_d[:])
            p2q_t = cpool.tile([128, 128], R, tag="p2q")
            nc.scalar.dma_start(p2q_t[:], p2q_d[:].bitcast(R))
            p2k_t = cpool.tile([64, 64], R, tag="p2k")
            nc.scalar.dma_start(p2k_t[:], p2k_d[:].bitcast(R))
            mh2_t = cpool.tile([128, 2], R, tag="mh2")
            nc.scalar.dma_start(mh2_t[:], mh2_d[:].bitcast(R))
            bch2_t = cpool.tile([2, 128], R, tag="bch2")
            nc.scalar.dma_start(bch2_t[:], bch2_d[:].bitcast(R))
            mh64_t = cpool.tile([64, 1], R, tag="mh64")
            nc.scalar.dma_start(mh64_t[:], mh64_d[:].bitcast(R))
            onesD_t = cpool.tile([128, 1], R, tag="onesD")
            nc.scalar.dma_start(onesD_t[:], onesD_d[:].bitcast(R))
            one1_t = cpool.tile([1, 2], R, tag="one1")
            nc.scalar.dma_start(one1_t[:], one1_d[:].bitcast(R))
            onesc_t = cpool.tile([128, 64], R, tag="onesc")
            nc.scalar.dma_start(onesc_t[:], onesc_d[:].bitcast(R))
            invgo_t = cpool.tile([128, L * 8], f32, tag="invgo")
            nc.scalar.dma_start(invgo_t[:], invgo_d[:])

            epsc = cpool.tile([128, 1], f32, tag="epsc")
            nc.gpsimd.memset(epsc[:], EPS)
            eps12 = cpool.tile([128, 1], f32, tag="eps12")
            nc.gpsimd.memset(eps12[:], 1e-12)

            # ---------- persistent activations ----------
            xt = rpool.tile([128, NMT * NP], R, tag="x")
            for mt in range(NMT):
                nc.gpsimd.dma_start(
                    xt[:, mt * NP:(mt + 1) * NP],
                    xT_d[mt * 128:(mt + 1) * 128, :].bitcast(R))
            o2xn = rpool.tile([128, NMT * NP], bf16, tag="o2xn")
            qhat = rpool.tile([128, 4 * NP], R, tag="qhat")
            khat = rpool.tile([128, NP], R, tag="khat")
            vaug = rpool.tile([128, 5 * 65], bf16, tag="vaug")
            oT = rpool.tile([64, H * NP], bf16, tag="oT")
            sff = rpool.tile([128, NMT * NP], bf16, tag="sff")
            m_sb = rpool.tile([1, NP], R, tag="m_sb")
            s_sb = rpool.tile([1, NP], R, tag="s_sb")
            r_sb = rpool.tile([1, NP], R, tag="r_sb")
            mo_sb = rpool.tile([1, NP], bf16, tag="mo_sb")
            for c in range(5):
                nc.gpsimd.dma_start(vaug[:, c * 65 + 64:c * 65 + 65], onesb_d[:])
            # ebias is large (4.3MB) and first needed by layer-0 attention:
            # issue after xT/layer-0 weight DMAs so they aren't starved.
            eb_t = cpool.tile([128, H * NSLOT * QT], bf16, tag="eb")
            nc.scalar.dma_start(eb_t[:], eb_d[:])

            def rot_l2(pq, np_, p2t, mht, qo, qw, dst):
                """rotary + l2norm(*4). pq: [np_, QT] PSUM -> dst (tile, off)."""
                qs = rotp.tile([128, QT], R, tag="qs")
                nc.scalar.activation(qs[0:np_, :], pq[:, :], AF.Copy)
                rps = ps.tile([128, QT], f32, tag="mm")
                nc.tensor.matmul(rps[0:np_, :], p2t[:], qs[0:np_, :],
                                 start=True, stop=True)
                t1 = rotp.tile([128, QT], R, tag="t1")
                nc.vector.scalar_tensor_tensor(
                    t1[0:np_, :], qs[0:np_, :], 1.0, cosx_t[0:np_, qo:qo + qw],
                    ALU.mult, ALU.mult)
                t2 = rotp.tile([128, QT], R, tag="t2")
                nc.vector.scalar_tensor_tensor(
                    t2[0:np_, :], rps[0:np_, :], 1.0, sinx_t[0:np_, qo:qo + qw],
                    ALU.mult, ALU.mult)
                qr = rotp.tile([128, QT], R, tag="qr")
                nc.gpsimd.tensor_tensor(qr[0:np_, :], t1[0:np_, :], t2[0:np_, :],
                                        ALU.add)
                sq = rotp.tile([128, QT], R, tag="sq")
                nc.scalar.activation(sq[0:np_, :], qr[0:np_, :], AF.Square)
                nh = np_ // 64
                ssq = psbc.tile([1, QT] if nh == 1 else [2, QT], f32, tag="bc")
                nc.tensor.matmul(ssq[:], mht[:], sq[0:np_, :], start=True, stop=True)
                sh = vecp.tile([2, QT], f32, tag="sh")
                nc.scalar.activation(sh[0:nh, :], ssq[:], AF.Sqrt,
                                     scale=1.0 / 16.0, bias=eps12[0:nh, :])
                rh = vecp.tile([2, QT], bf16, tag="rh")
                nc.vector.reciprocal(rh[0:nh, :], sh[0:nh, :])
                dt_, off = dst
                if nh == 1:
                    bcq = rotp.tile([128, QT], R, tag="bcq")
                    nc.gpsimd.partition_broadcast(bcq[0:64, :].bitcast(f32), rh[0:1, :].bitcast(f32), 64)
                    nc.vector.scalar_tensor_tensor(
                        dt_[0:np_, off:off + qw], qr[0:np_, :], 1.0,
                        bcq[0:np_, :], ALU.mult, ALU.mult)
                else:
                    bcp = psbc.tile([128, QT], f32, tag="bc")
                    nc.tensor.matmul(bcp[:], bch2_t[:], rh[0:nh, :],
                                     start=True, stop=True)
                    nc.vector.scalar_tensor_tensor(
                        dt_[0:np_, off:off + qw], qr[0:np_, :], 1.0,
                        bcp[0:np_, :], ALU.mult, ALU.mult)

            # ================= layers =================
            for l in range(L):
                # ---- per-layer small weights ----
                wkt = wsp.tile([128, 8 * DH], R, tag="wk")
                nc.gpsimd.dma_start(
                    wkt[:].rearrange("p (c m) -> p c m", c=8), wk_d[l].bitcast(R))
                wvt = wsp.tile([128, 8 * DH], R, tag="wv")
                nc.gpsimd.dma_start(
                    wvt[:].rearrange("p (c m) -> p c m", c=8), wv_d[l].bitcast(R))
                kvec_t = wsp.tile([1, 4 * DH], R, tag="kvec")
                nc.gpsimd.dma_start(kvec_t[:], kvec_d[l].bitcast(R))
                wk1n_t = kvec_t[0:1, 0:DH]
                bkp_t = kvec_t[0:1, DH:2 * DH]
                wv1n_t = kvec_t[0:1, 2 * DH:3 * DH]
                bvv_t = kvec_t[0:1, 3 * DH:4 * DH]
                wq1n_t = wsp.tile([1, 4 * 128], R, tag="wq1n")
                nc.gpsimd.dma_start(wq1n_t[:], wq1n_d[l].bitcast(R))
                wo1n_t = wsp.tile([64, 8], bf16, tag="wo1n")
                nc.gpsimd.dma_start(wo1n_t[:], wo1n_d[l])
                gorow_t = wsp.tile([1, D], bf16, tag="gorow", bufs=1)
                nc.gpsimd.dma_start(gorow_t[:], gorow_d[0:1, l * D:(l + 1) * D])

                # ---- A: attn stats (m, s, r per token) ----
                for (qo, qw) in QTS:
                    s1 = psbc.tile([1, QT], f32, tag="bc")
                    s2 = psbc.tile([1, QT], f32, tag="bc")
                    for mt in range(NMT):
                        seg = xt[:, mt * NP + qo:mt * NP + qo + qw]
                        sq = scrp.tile([128, QT], R, tag="sq")
                        nc.scalar.activation(sq[:], seg, AF.Square)
                        nc.tensor.matmul(s1[:], onesD_t[:], seg,
                                         start=(mt == 0), stop=(mt == NMT - 1))
                        nc.tensor.matmul(s2[:], onesD_t[:], sq[:],
                                         start=(mt == 0), stop=(mt == NMT - 1))
                    nc.vector.tensor_copy(m_sb[0:1, qo:qo + qw], s1[:])
                    msq = vecp.tile([1, QT], f32, tag="msq")
                    nc.scalar.activation(msq[:], s1[:], AF.Square)
                    v_v = vecp.tile([1, QT], f32, tag="v")
                    nc.vector.scalar_tensor_tensor(v_v[:], s2[:], 1.0, msq[:],
                                                   ALU.mult, ALU.subtract)
                    nc.scalar.activation(s_sb[0:1, qo:qo + qw], v_v[:], AF.Sqrt,
                                         bias=epsc[0:1, :])
                    nc.vector.reciprocal(r_sb[0:1, qo:qo + qw],
                                         s_sb[0:1, qo:qo + qw])

                # ---- B/C/D: K, Q, V -- mains pipelined ahead of the
                # stats-dependent finishers (corrections + rotary/l2norm) ----
                def k_main(qi):
                    qo, qw = QTS[qi]
                    pk = ps.tile([64, QT], f32, tag="mm")
                    for c in range(8):
                        nc.tensor.matmul(pk[:], wkt[:, c * DH:(c + 1) * DH],
                                         xt[:, c * NP + qo:c * NP + qo + qw],
                                         start=(c == 0), stop=False)
                    return pk

                def k_fin(qi, pk):
                    qo, qw = QTS[qi]
                    nc.tensor.matmul(pk[:], wk1n_t, m_sb[0:1, qo:qo + qw],
                                     start=False, stop=False)
                    nc.tensor.matmul(pk[:], bkp_t, s_sb[0:1, qo:qo + qw],
                                     start=False, stop=True)
                    rot_l2(pk, 64, p2k_t, mh64_t, qo, qw, (khat, qo))

                def q_main(p, wqt, qi):
                    qo, qw = QTS[qi]
                    pq = ps.tile([128, QT], f32, tag="mm")
                    for c in range(8):
                        nc.tensor.matmul(pq[:], wqt[:, c * 128:(c + 1) * 128],
                                         xt[:, c * NP + qo:c * NP + qo + qw],
                                         start=(c == 0), stop=False)
                    return pq

                def q_fin(p, qi, pq):
                    qo, qw = QTS[qi]
                    nc.tensor.matmul(pq[:], wq1n_t[0:1, p * 128:(p + 1) * 128],
                                     m_sb[0:1, qo:qo + qw],
                                     start=False, stop=True)
                    rot_l2(pq, 128, p2q_t, mh2_t, qo, qw, (qhat, p * NP + qo))

                def v_chunk(t):
                    to, tw = TCH[t]
                    rtk = psbc.tile([128, 2], f32, tag="bc")
                    nc.tensor.matmul(rtk[0:tw, :], r_sb[0:1, to:to + tw],
                                     one1_t[:], start=True, stop=True)
                    pv = ps.tile([128, DH], f32, tag="mm")
                    for c in range(8):
                        nc.tensor.matmul(pv[0:tw, :],
                                         xt[:, c * NP + to:c * NP + to + tw],
                                         wvt[:, c * DH:(c + 1) * DH],
                                         start=(c == 0), stop=False)
                    nc.tensor.matmul(pv[0:tw, :], m_sb[0:1, to:to + tw],
                                     wv1n_t, start=False, stop=False)
                    nc.tensor.matmul(pv[0:tw, :], s_sb[0:1, to:to + tw],
                                     bvv_t, start=False, stop=True)
                    with tc.high_priority():
                        rts = vecp.tile([128, 1], f32, tag="rts")
                        nc.vector.tensor_copy(rts[0:tw, :], rtk[0:tw, 0:1])
                        nc.scalar.activation(vaug[0:tw, t * 65:t * 65 + DH],
                                             pv[0:tw, :], AF.Copy,
                                             scale=rts[0:tw, :])

                wqts = {}
                def load_wq(p):
                    wqt = wqp.tile([128, 8 * 128], R, tag="wq")
                    nc.sync.dma_start(
                        wqt[:].rearrange("p (c m) -> p c m", c=8),
                        wq_d[l, p].bitcast(R))
                    wqts[p] = wqt

                load_wq(0)
                pk0 = k_main(0)
                pk1 = k_main(1)
                load_wq(1)
                pq = {}
                pq[(0, 0)] = q_main(0, wqts[0], 0)
                k_fin(0, pk0)
                pq[(0, 1)] = q_main(0, wqts[0], 1)
                k_fin(1, pk1)
                nc.gpsimd.dma_start(khat[64:128, 0:NP], khat[0:64, 0:NP])
                nc.gpsimd.dma_start(khat[:, NK:NK + 1], nullk_d[l])
                load_wq(2)
                pq[(1, 0)] = q_main(1, wqts[1], 0)
                q_fin(0, 0, pq[(0, 0)])
                pq[(1, 1)] = q_main(1, wqts[1], 1)
                q_fin(0, 1, pq[(0, 1)])
                load_wq(3)
                pq[(2, 0)] = q_main(2, wqts[2], 0)
                q_fin(1, 0, pq[(1, 0)])
                pq[(2, 1)] = q_main(2, wqts[2], 1)
                q_fin(1, 1, pq[(1, 1)])
                pq[(3, 0)] = q_main(3, wqts[3], 0)
                q_fin(2, 0, pq[(2, 0)])
                pq[(3, 1)] = q_main(3, wqts[3], 1)
                q_fin(2, 1, pq[(2, 1)])
                v_chunk(0)
                v_chunk(1)
                v_chunk(2)
                q_fin(3, 0, pq[(3, 0)])
                v_chunk(3)
                v_chunk(4)
                q_fin(3, 1, pq[(3, 1)])
                # null v at key row NK=515 -> chunk 4, partition 3
                nc.gpsimd.dma_start(vaug[3:4, 4 * 65:4 * 65 + DH], nullv_d[l])

                # ---- E: attention (unnormalized softmax) ----
                for h in range(H):
                    hb = 64 * (h & 1)
                    qcol = (h >> 1) * NP
                    for qi, (qo, qw) in enumerate(QTS):
                        av = psacc.tile([128, QT], f32, tag="av")
                        chunks = CHUNKS[qi]
                        for ci, c in enumerate(chunks):
                            kw = KW[c]
                            slot = (ci if qi == 0 else 4 + ci)
                            sp = ps.tile([128, QT], f32, tag="mm")
                            nc.tensor.matmul(
                                sp[0:kw, :],
                                khat[hb:hb + 64, 128 * c:128 * c + kw],
                                qhat[hb:hb + 64, qcol + qo:qcol + qo + qw],
                                start=True, stop=True)
                            aex = attp.tile([128, QT], bf16, tag="aex")
                            nc.scalar.activation(aex[0:kw, :], sp[0:kw, :], AF.Exp)
                            au = attp.tile([128, QT], bf16, tag="au")
                            ebs = (h * NSLOT + slot) * QT
                            with tc.high_priority():
                                nc.vector.scalar_tensor_tensor(
                                    au[0:kw, :], aex[0:kw, :], 1.0,
                                    eb_t[0:kw, ebs:ebs + QT], ALU.mult, ALU.mult)
                            nc.tensor.matmul(av[0:65, :],
                                             vaug[0:kw, c * 65:(c + 1) * 65],
                                             au[0:kw, :], start=(ci == 0),
                                             stop=(ci == len(chunks) - 1))
                        rd = scrp.tile([128, QT], R, tag="rd", bufs=2)
                        nc.vector.reciprocal(rd[64:65, :], av[64:65, :])
                        bco = psbc.tile([64, QT], f32, tag="bc")
                        nc.tensor.matmul(bco[0:64, :], onesc_t[64:65, :],
                                         rd[64:65, :], start=True, stop=True)
                        bcos = attp.tile([64, QT], bf16, tag="bcos", bufs=2)
                        nc.vector.tensor_copy(bcos[:], bco[0:64, :])
                        nc.vector.scalar_tensor_tensor(
                            oT[0:64, h * NP + qo:h * NP + qo + qw],
                            av[0:64, :], 1.0, bcos[:], ALU.mult, ALU.mult)

                # ---- F: Wo + out-LN + residual ----
                wots = []
                for mt in range(NMT):
                    wot = wop.tile([64, 8 * 128], bf16, tag="wo")
                    nc.gpsimd.dma_start(
                        wot[:].rearrange("p (c m) -> p c m", c=8), wo_d[l, mt])
                    wots.append(wot)
                for (qo, qw) in QTS:
                    mo = psbc.tile([1, QT], f32, tag="bc")
                    for c in range(8):
                        nc.tensor.matmul(mo[:], wo1n_t[:, c:c + 1],
                                         oT[0:64, c * NP + qo:c * NP + qo + qw],
                                         start=(c == 0), stop=(c == 7))
                    nc.vector.tensor_copy(mo_sb[0:1, qo:qo + qw], mo[:])
                    s2c = psbc.tile([1, QT], f32, tag="bc")
                    for mt in range(NMT):
                        pl = ps.tile([128, QT], f32, tag="mm")
                        for c in range(8):
                            nc.tensor.matmul(pl[:],
                                             wots[mt][:, c * 128:(c + 1) * 128],
                                             oT[0:64, c * NP + qo:c * NP + qo + qw],
                                             start=(c == 0), stop=False)
                        nc.tensor.matmul(pl[:],
                                         gorow_t[0:1, mt * 128:(mt + 1) * 128],
                                         mo_sb[0:1, qo:qo + qw],
                                         start=False, stop=True)
                        sqo = scrp.tile([128, QT], R, tag="sq")
                        nc.scalar.activation(sqo[:], pl[:], AF.Square,
                                             scale=invgo_t[:, l * 8 + mt:
                                                           l * 8 + mt + 1])
                        nc.tensor.matmul(s2c[:], onesD_t[:], sqo[:],
                                         start=(mt == 0), stop=(mt == NMT - 1))
                        nc.scalar.activation(
                            o2xn[:, mt * NP + qo:mt * NP + qo + qw],
                            pl[:], AF.Copy)
                    so = vecp.tile([1, QT], f32, tag="so")
                    nc.scalar.activation(so[:], s2c[:], AF.Sqrt, bias=epsc[0:1, :])
                    ro = vecp.tile([1, QT], bf16, tag="ro")
                    nc.vector.reciprocal(ro[:], so[:])
                    rb = sgp.tile([128, QT], f32, tag="rbb", bufs=2)
                    nc.gpsimd.partition_broadcast(rb[:], ro[:], 128)
                    for mt in range(NMT):
                        eng = nc.vector if mt % 2 == 0 else nc.gpsimd
                        tt = scrp.tile([128, QT], bf16, tag="tt")
                        eng.tensor_tensor(
                            tt[:], o2xn[:, mt * NP + qo:mt * NP + qo + qw],
                            rb[:], ALU.mult)
                        xcols = xt[:, mt * NP + qo:mt * NP + qo + qw]
                        eng.tensor_tensor(xcols, xcols, tt[:], ALU.add)

                # ---- G: ff-LN -> xn (per qtile, interleaved with FFN
                # block 0 so PE keeps working during the q1 stats chain) ----
                def ff_ln(qi):
                    qo, qw = QTS[qi]
                    s1 = psbc.tile([1, QT], f32, tag="bc")
                    s2 = psbc.tile([1, QT], f32, tag="bc")
                    for mt in range(NMT):
                        seg = xt[:, mt * NP + qo:mt * NP + qo + qw]
                        sq = scrp.tile([128, QT], R, tag="sq")
                        nc.scalar.activation(sq[:], seg, AF.Square)
                        nc.tensor.matmul(s1[:], onesD_t[:], seg,
                                         start=(mt == 0), stop=(mt == NMT - 1))
                        nc.tensor.matmul(s2[:], onesD_t[:], sq[:],
                                         start=(mt == 0), stop=(mt == NMT - 1))
                    msq = vecp.tile([1, QT], f32, tag="vf")
                    nc.scalar.activation(msq[:], s1[:], AF.Square)
                    v_v = vecp.tile([1, QT], f32, tag="vf")
                    nc.vector.scalar_tensor_tensor(v_v[:], s2[:], 1.0, msq[:],
                                                   ALU.mult, ALU.subtract)
                    sf = vecp.tile([1, QT], f32, tag="vf")
                    nc.scalar.activation(sf[:], v_v[:], AF.Sqrt, bias=epsc[0:1, :])
                    rf = vecp.tile([1, QT], f32, tag="vbr")
                    nc.vector.reciprocal(rf[:], sf[:])
                    mrv = vecp.tile([1, QT], f32, tag="vbr")
                    nc.vector.scalar_tensor_tensor(mrv[:], s1[:], 1.0, rf[:],
                                                   ALU.mult, ALU.mult)
                    rb = sgp.tile([128, QT], f32, tag="rbb", bufs=2)
                    nc.gpsimd.partition_broadcast(rb[:], rf[:], 128)
                    mrb = sgp.tile([128, QT], f32, tag="mrbb", bufs=2)
                    nc.gpsimd.partition_broadcast(mrb[:], mrv[:], 128)
                    for mt in range(NMT):
                        eng = nc.vector if mt % 2 == 0 else nc.gpsimd
                        tt = scrp.tile([128, QT], bf16, tag="tt")
                        eng.tensor_tensor(
                            tt[:], xt[:, mt * NP + qo:mt * NP + qo + qw],
                            rb[:], ALU.mult)
                        eng.tensor_tensor(
                            o2xn[:, mt * NP + qo:mt * NP + qo + qw],
                            tt[:], mrb[:], ALU.subtract)

                # ---- H: FFN (SwiGLU), 4 halves of 8 s-blocks ----
                def ffn1_qt(mi, wga, wgg, qi):
                    qo, qw = QTS[qi]
                    pg = ps.tile([128, QT], f32, tag="mm")
                    for c in range(8):
                        nc.tensor.matmul(pg[:], wgg[:, c * 128:(c + 1) * 128],
                                         o2xn[:, c * NP + qo:c * NP + qo + qw],
                                         start=(c == 0), stop=(c == 7))
                    sig = sgp.tile([128, QT], bf16, tag="sig")
                    nc.scalar.activation(sig[:], pg[:], AF.Sigmoid)
                    pa = ps.tile([128, QT], f32, tag="mm")
                    for c in range(8):
                        nc.tensor.matmul(pa[:], wga[:, c * 128:(c + 1) * 128],
                                         o2xn[:, c * NP + qo:c * NP + qo + qw],
                                         start=(c == 0), stop=(c == 7))
                    gs = sgp.tile([128, QT], bf16, tag="ag")
                    nc.vector.scalar_tensor_tensor(
                        gs[:], pg[:], 1.0, sig[:], ALU.mult, ALU.mult)
                    nc.vector.scalar_tensor_tensor(
                        sff[:, mi * NP + qo:mi * NP + qo + qw],
                        pa[:], 1.0, gs[:], ALU.mult, ALU.mult)

                def load_wff1(m):
                    wga = wf1p.tile([128, 8 * 128], bf16, tag="wff1")
                    nc.sync.dma_start(
                        wga[:].rearrange("p (c m) -> p c m", c=8),
                        wff1_d[l, m, 0])
                    wgg = wf1p.tile([128, 8 * 128], bf16, tag="wff1")
                    nc.sync.dma_start(
                        wgg[:].rearrange("p (c m) -> p c m", c=8),
                        wff1_d[l, m, 1])
                    return wga, wgg

                ff_ln(0)
                w00 = load_wff1(0)
                ffn1_qt(0, w00[0], w00[1], 0)
                ff_ln(1)
                ffn1_qt(0, w00[0], w00[1], 1)
                for half in range(4):
                    for mi in range(8):
                        if half == 0 and mi == 0:
                            continue
                        m = half * 8 + mi
                        wga, wgg = load_wff1(m)
                        for qi in range(2):
                            ffn1_qt(mi, wga, wgg, qi)
                    for mt in range(NMT):
                        w2 = wf2p.tile([128, 8 * 128], bf16, tag="wff2")
                        nc.sync.dma_start(
                            w2[:].rearrange("p (c m) -> p c m", c=8),
                            wff2_d[l, half, mt])
                        for (qo, qw) in QTS:
                            pl = ps.tile([128, QT], f32, tag="mm")
                            for c in range(8):
                                nc.tensor.matmul(
                                    pl[:], w2[:, c * 128:(c + 1) * 128],
                                    sff[:, c * NP + qo:c * NP + qo + qw],
                                    start=(c == 0), stop=(c == 7))
                            xcols = xt[:, mt * NP + qo:mt * NP + qo + qw]
                            nc.gpsimd.tensor_tensor(xcols, xcols, pl[:], ALU.add)

            # ================= final stable LN + Wproj =================
            xm = cpool.tile([128, NP], bf16, tag="xm")
            nc.vector.tensor_tensor(xm[:], xt[:, 0:NP], xt[:, NP:2 * NP], ALU.max)
            for mt in range(2, NMT):
                nc.vector.tensor_tensor(xm[:], xm[:], xt[:, mt * NP:(mt + 1) * NP],
                                        ALU.max)
            mxb = cpool.tile([128, NP], bf16, tag="mxb")
            from concourse import bass_isa
            nc.gpsimd.partition_all_reduce(mxb[:], xm[:], 128, bass_isa.ReduceOp.max)

            for (qo, qw) in QTS:
                s1 = psbc.tile([1, QT], f32, tag="bc")
                s2 = psbc.tile([1, QT], f32, tag="bc")
                for mt in range(NMT):
                    seg = xt[:, mt * NP + qo:mt * NP + qo + qw]
                    sq = scrp.tile([128, QT], R, tag="sq")
                    nc.scalar.activation(sq[:], seg, AF.Square)
                    nc.tensor.matmul(s1[:], onesD_t[:], seg,
                                     start=(mt == 0), stop=(mt == NMT - 1))
                    nc.tensor.matmul(s2[:], onesD_t[:], sq[:],
                                     start=(mt == 0), stop=(mt == NMT - 1))
                msq = vecp.tile([1, QT], f32, tag="msq")
                nc.scalar.activation(msq[:], s1[:], AF.Square)
                v_v = vecp.tile([1, QT], f32, tag="v")
                nc.vector.scalar_tensor_tensor(v_v[:], s2[:], 1.0, msq[:],
                                               ALU.mult, ALU.subtract)
                mxsq = vecp.tile([1, QT], f32, tag="mxsq")
                nc.scalar.activation(mxsq[:], mxb[0:1, qo:qo + qw], AF.Square)
                veps = vecp.tile([1, QT], f32, tag="veps")
                nc.vector.scalar_tensor_tensor(veps[:], mxsq[:], EPS, v_v[:],
                                               ALU.mult, ALU.add)
                sf = vecp.tile([1, QT], f32, tag="sf")
                nc.scalar.activation(sf[:], veps[:], AF.Sqrt)
                rf = vecp.tile([1, QT], bf16, tag="rf")
                nc.vector.reciprocal(rf[:], sf[:])
                mrv = vecp.tile([1, QT], bf16, tag="mrv")
                nc.vector.scalar_tensor_tensor(mrv[:], s1[:], 1.0, rf[:],
                                               ALU.mult, ALU.mult)
                rb = sgp.tile([128, QT], bf16, tag="rbb")
                nc.gpsimd.partition_broadcast(rb[:], rf[:], 128)
                mrb = sgp.tile([128, QT], bf16, tag="mrbb")
                nc.gpsimd.partition_broadcast(mrb[:], mrv[:], 128)
                for mt in range(NMT):
                    tt = scrp.tile([128, QT], bf16, tag="tt")
                    nc.vector.scalar_tensor_tensor(
                        tt[:], xt[:, mt * NP + qo:mt * NP + qo + qw], 1.0,
                        rb[:], ALU.mult, ALU.mult)
                    nc.vector.scalar_tensor_tensor(
                        o2xn[:, mt * NP + qo:mt * NP + qo + qw],
                        tt[:], 1.0, mrb[:], ALU.mult, ALU.subtract)

            for half in range(2):
                wps = []
                for c in range(8):
                    wp = wpp.tile([128, 512], bf16, tag=f"wp{c}")
                    nc.sync.dma_start(wp[:], wproj_d[half, c])
                    wps.append(wp)
                for t, (to, tw) in enumerate(TCH):
                    rtw = min(tw, max(0, N - to))
                    if rtw == 0:
                        continue
                    pn = psacc.tile([128, 512], f32, tag="av")
                    for c in range(8):
                        nc.tensor.matmul(pn[0:tw, :],
                                         o2xn[:, c * NP + to:c * NP + to + tw],
                                         wps[c][:], start=(c == 0), stop=(c == 7))
                    st = scrp.tile([128, 512], f32, tag="outst", bufs=2)
                    nc.scalar.activation(st[0:rtw, :], pn[0:rtw, :], AF.Copy)
                    nc.sync.dma_start(out_d[to:to + rtw, half * 512:(half + 1) * 512],
                                      st[0:rtw, :])

    nc.compile()
    return nc


_CACHE = {}


def _get_program():
    if 'nc' not in _CACHE:
        _CACHE['nc'] = _build()
    return _CACHE['nc']


def kernel(**inputs) -> np.ndarray:
    from concourse.bass_utils import run_bass_kernel_spmd
    host = _host_prep(inputs)
    nc = _get_program()
    shared = {k: v for k, v in host.items() if k != 'xT'}
    in_maps = [dict(shared, xT=np.ascontiguousarray(host['xT'][b])) for b in range(B)]
    res = run_bass_kernel_spmd(nc, in_maps, list(range(B)))
    out = np.stack([res.results[b]['out'] for b in range(B)], axis=0)
    _CACHE['last_results'] = res
    return out


# revision 72
# speedup vs baseline: 1.0116x; 1.0012x over previous
"""Trainium2 Bass kernel for nn_BasePriorNetwork (4-layer dense transformer).

Sharding: data-parallel over batch (B=8) across 8 NeuronCores; weights
replicated (bf16). Activations feature-major [feat, token]; residual kept
f32. Key algebraic restructurings vs a straightforward lowering:
  - attn-LN is never materialized: l2norm makes Q/K invariant to the
    per-token rsqrt(var) scale, so Q/K/V consume the raw residual with a
    rank-1 mean-correction matmul; K's bias is pre-divided by the scale via
    a sqrt(var)-weighted rank-1 term; V is rescaled per token by r inside
    the PSUM->SBUF copy (Act scale).
  - softmax is left unnormalized: the post-Wo LayerNorm is invariant to a
    per-token positive scale, so the 1/sum(exp) divide cancels exactly.
  - rel-pos bias + causal mask are folded into a precomputed exp(bias)
    multiplier (exact zeros on masked entries).
  - rotary is applied as q*cosx + (P2@q)*sinx with a constant permutation
    matrix P2 on the PE, processing two heads per op.
  - all LN gains are folded into the adjacent weight matrices host-side.
"""
import sys, math
sys.path.insert(0, '/opt/trn_rl_repo')
import numpy as np
import ml_dtypes

import concourse.bass as bass
import concourse.bacc as bacc
import concourse.tile as tile
from concourse import mybir

f32 = mybir.dt.float32
f32r = mybir.dt.float32r
bf16 = mybir.dt.bfloat16
AF = mybir.ActivationFunctionType
ALU = mybir.AluOpType

B, N, D = 8, 515, 1024
H, DH, L = 8, 64, 4
FF = 4 * D
ROT = 32
NB, MAXD = 32, 128
EPS = 1e-5

NP = 520                      # padded tokens
QT = 260                      # query/free tile (2 per NP)
QTS = [(0, QT), (QT, QT)]
NK = 515                      # key row of the null key (= token rows + 1)
NMT = 8                       # feature tiles per 1024
TCH = [(0, 128), (128, 128), (256, 128), (384, 128), (512, 8)]
# chunks of key rows per qtile (causal-trimmed; chunk 4 holds the null key)
CHUNKS = [[0, 1, 2, 4], [0, 1, 2, 3, 4]]
NSLOT = 9
KW = [128, 128, 128, 128, 8]

# head-dim permutation: rows 0:16 even rot dims, 16:32 pass, 32:48 odd rot
# dims, 48:64 pass.
PERM = (list(range(0, ROT, 2)) + list(range(ROT, ROT + 16))
        + list(range(1, ROT, 2)) + list(range(ROT + 16, DH)))

BF = ml_dtypes.bfloat16


def _np_bias(emb):
    """bias[h, i, j'] as in reference (i query 0..N-1, j'=0 null, j'=t+1)."""
    q_pos = np.arange(N)
    k_pos = np.arange(N + 1)
    rel = k_pos[None, :] - q_pos[:, None]
    nn = np.maximum(-rel, 0)
    max_exact = NB // 2
    is_small = nn < max_exact
    nf = np.maximum(nn, 1).astype(np.float32)
    val_large = max_exact + (
        np.log(nf / np.float32(max_exact)).astype(np.float32)
        / np.float32(math.log(MAXD / max_exact)) * np.float32(NB - max_exact)
    ).astype(np.int32)
    val_large = np.minimum(val_large, NB - 1)
    bucket = np.where(is_small, nn, val_large)          # [N, N+1]
    return np.transpose(emb[bucket], (2, 0, 1)).astype(np.float32)  # [H,N,N+1]


def _np_ebias(emb):
    """ebias[h, keyrow, qcol] = exp(bias) with causal+pad mask folded (0).
    Key row t = token t; null key at row NK=515."""
    bias = _np_bias(emb)                                 # [H, N, N+1]
    eb = np.zeros((H, NP, NP), np.float32)
    t = np.arange(N)
    vals = np.exp(bias[:, :, 1:])                        # [H, i, t]
    allowed = (t[None, :] <= np.arange(N)[:, None])      # t <= i
    vals = vals * allowed[None, :, :]
    eb[:, :N, :N] = np.transpose(vals, (0, 2, 1))        # [H, keyrow, i]
    eb[:, NK, :N] = np.exp(bias[:, :, 0])                # null row per query i
    eb[:, NK, N:] = 1.0                                  # padded queries
    # slot layout [128, H, NSLOT, QT]
    out = np.zeros((128, H, NSLOT, QT), np.float32)
    for qi in range(2):
        for ci, c in enumerate(CHUNKS[qi]):
            s = ci if qi == 0 else 4 + ci
            kw = KW[c]
            out[:kw, :, s, :] = np.transpose(
                eb[:, 128 * c:128 * c + kw, qi * QT:(qi + 1) * QT], (1, 0, 2))
    return out.astype(BF)


def _host_prep(inputs):
    f = np.float32
    x = np.asarray(inputs['x'], f)
    Wq = np.asarray(inputs['Wq'], f)
    Wkv = np.asarray(inputs['Wkv'], f)
    bkv = np.asarray(inputs['bkv'], f)
    null_kv = np.asarray(inputs['null_kv'], f)
    Wo = np.asarray(inputs['Wo'], f)
    ga_all = np.asarray(inputs['attn_norm_g'], f)
    go_all = np.asarray(inputs['out_norm_g'], f)
    gf_all = np.asarray(inputs['ff_norm_g'], f)
    Wff1 = np.asarray(inputs['Wff1'], f)
    Wff2 = np.asarray(inputs['Wff2'], f)
    gfin = np.asarray(inputs['final_norm_g'], f)
    Wproj = np.asarray(inputs['Wproj'], f)

    d = {}
    xT = np.zeros((B, D, NP), f)
    xT[:, :, :N] = np.transpose(x, (0, 2, 1))
    d['xT'] = xT

    d['ebias'] = np.ascontiguousarray(
        _np_ebias(np.asarray(inputs['relpos_emb'], f)).reshape(128, H * NSLOT * QT))

    # rotary tables
    inv_freq = (1.0 / (10000.0 ** (np.arange(0, ROT, 2, dtype=f) / ROT))).astype(f)
    th = np.arange(NP, dtype=f)[None, :] * inv_freq[:, None]     # [16, NP]
    cos, sin = np.cos(th).astype(f), np.sin(th).astype(f)
    one16, zero16 = np.ones_like(cos), np.zeros_like(cos)
    cosx = np.concatenate([cos, one16, cos, one16], 0)           # [64, NP]
    sinx = np.concatenate([-sin, zero16, sin, zero16], 0)
    d['cosx'] = np.ascontiguousarray(np.concatenate([cosx, cosx], 0)).astype(BF)
    d['sinx'] = np.ascontiguousarray(np.concatenate([sinx, sinx], 0)).astype(BF)

    p2 = np.zeros((64, 64), f)
    for i in range(16):
        p2[32 + i, i] = 1.0      # out even slot <- in odd row
        p2[i, 32 + i] = 1.0      # out odd slot <- in even row
    d['p2k'] = p2
    p2q = np.zeros((128, 128), f)
    p2q[:64, :64] = p2
    p2q[64:, 64:] = p2
    d['p2q'] = p2q

    mh2 = np.zeros((128, 2), f)
    mh2[:64, 0] = 1.0
    mh2[64:, 1] = 1.0
    d['mh2'] = mh2
    bch2 = np.zeros((2, 128), f)
    bch2[0, :64] = 1.0
    bch2[1, 64:] = 1.0
    d['bch2'] = bch2
    d['mh64'] = np.ones((64, 1), f)
    d['ones1x64'] = np.ones((1, 64), f).astype(BF)
    d['ones1x128'] = np.ones((1, 128), f).astype(BF)
    d['onesD'] = np.full((128, 1), 1.0 / D, f)
    d['one1'] = np.ones((1, 2), f)
    d['ones_c'] = np.ones((128, 64), f)
    d['onesb'] = np.ones((128, 1), f).astype(BF)

    go_safe = np.where(np.abs(go_all) < 1e-12, 1.0, go_all)
    d['invgo'] = np.ascontiguousarray(
        (1.0 / go_safe).reshape(L * 8, 128).T.astype(f))          # [128, L*8] f32
    d['gorow'] = np.ascontiguousarray(go_all.reshape(1, L * D)).astype(BF)

    # per-layer folded weights
    wq = np.zeros((L, 4, 128, 8, 128), f)
    wq1n = np.zeros((L, 4, 128), f)
    wk = np.zeros((L, 128, 8, DH), f)
    wv = np.zeros((L, 128, 8, DH), f)
    kvec = np.zeros((L, 1, 4 * DH), f)   # wk1n | bkp | wv1n | bvv
    nullk = np.zeros((L, 128, 1), f)
    nullv = np.zeros((L, 1, DH), f)
    wo = np.zeros((L, 8, 64, 8, 128), f)
    wo1n = np.zeros((L, 64, 8), f)
    wff1 = np.zeros((L, 32, 2, 128, 8, 128), f)
    wff2 = np.zeros((L, 4, 8, 128, 8, 128), f)
    for l in range(L):
        ga, go, gf = ga_all[l], go_all[l], gf_all[l]
        Wq_g = ga[:, None] * Wq[l]                                # [D, 512]
        for p in range(4):
            pw = np.concatenate(
                [Wq_g[:, (2 * p) * DH:(2 * p + 1) * DH][:, PERM],
                 Wq_g[:, (2 * p + 1) * DH:(2 * p + 2) * DH][:, PERM]], 1)
            wq[l, p] = pw.reshape(8, 128, 128).transpose(1, 0, 2)
            wq1n[l, p] = -pw.sum(0)
        Wk_g = (ga[:, None] * Wkv[l][:, :DH])[:, PERM]
        wk[l] = Wk_g.reshape(8, 128, DH).transpose(1, 0, 2)
        Wv_g = ga[:, None] * Wkv[l][:, DH:]
        wv[l] = Wv_g.reshape(8, 128, DH).transpose(1, 0, 2)
        kvec[l, 0, 0:DH] = -Wk_g.sum(0)
        kvec[l, 0, DH:2 * DH] = bkv[l, :DH][PERM]
        kvec[l, 0, 2 * DH:3 * DH] = -Wv_g.sum(0)
        kvec[l, 0, 3 * DH:] = bkv[l, DH:]
        kn = null_kv[l, 0][PERM]
        kh = 4.0 * kn / max(np.linalg.norm(kn), 1e-12)
        nullk[l, :64, 0] = kh
        nullk[l, 64:, 0] = kh
        nullv[l, 0] = null_kv[l, 1]
        Wo_g = Wo[l] * go[None, :]                                # [512, D]
        wo[l] = Wo_g.reshape(8, 64, 8, 128).transpose(2, 1, 0, 3)
        wo1n[l] = -(Wo[l].sum(1) / D).reshape(8, 64).T
        Wf1_g = gf[:, None] * Wff1[l]                             # [D, 2FF]
        a = Wf1_g[:, :FF].reshape(8, 128, 32, 128)
        g = Wf1_g[:, FF:].reshape(8, 128, 32, 128)
        wff1[l, :, 0] = a.transpose(2, 1, 0, 3)
        wff1[l, :, 1] = g.transpose(2, 1, 0, 3)
        wff2[l] = Wff2[l].reshape(4, 8, 128, 8, 128).transpose(0, 3, 2, 1, 4)
    d['wq'] = wq
    d['wq1n'] = np.ascontiguousarray(wq1n.reshape(L, 1, 4 * 128))
    d['wk'] = wk
    d['wv'] = wv
    d['kvec'] = kvec
    d['nullk'] = nullk.astype(BF)
    d['nullv'] = nullv.astype(BF)
    d['wo'] = wo.astype(BF)
    d['wo1n'] = wo1n.astype(BF)
    d['wff1'] = wff1.astype(BF)
    d['wff2'] = wff2.astype(BF)
    Wp_g = gfin[:, None] * Wproj
    d['wproj'] = np.ascontiguousarray(
        Wp_g.reshape(8, 128, 2, 512).transpose(2, 0, 1, 3)).astype(BF)
    return d


def _build():
    nc = bacc.Bacc("TRN2", target_bir_lowering=False, debug=False, num_devices=8)

    def P(name, shape, dt=bf16):
        return nc.declare_dram_parameter(name, list(shape), dt, isOutput=False)

    xT_d = P('xT', [D, NP], f32)
    eb_d = P('ebias', [128, H * NSLOT * QT])
    cosx_d = P('cosx', [128, NP])
    sinx_d = P('sinx', [128, NP])
    p2q_d = P('p2q', [128, 128], f32)
    p2k_d = P('p2k', [64, 64], f32)
    mh2_d = P('mh2', [128, 2], f32)
    bch2_d = P('bch2', [2, 128], f32)
    mh64_d = P('mh64', [64, 1], f32)
    o1x64_d = P('ones1x64', [1, 64])
    o1x128_d = P('ones1x128', [1, 128])
    onesD_d = P('onesD', [128, 1], f32)
    one1_d = P('one1', [1, 2], f32)
    onesb_d = P('onesb', [128, 1])
    onesc_d = P('ones_c', [128, 64], f32)
    invgo_d = P('invgo', [128, L * 8], f32)
    gorow_d = P('gorow', [1, L * D])
    wq_d = P('wq', [L, 4, 128, 8, 128], f32)
    wq1n_d = P('wq1n', [L, 1, 4 * 128], f32)
    wk_d = P('wk', [L, 128, 8, DH], f32)
    wv_d = P('wv', [L, 128, 8, DH], f32)
    kvec_d = P('kvec', [L, 1, 4 * DH], f32)
    nullk_d = P('nullk', [L, 128, 1])
    nullv_d = P('nullv', [L, 1, DH])
    wo_d = P('wo', [L, 8, 64, 8, 128])
    wo1n_d = P('wo1n', [L, 64, 8])
    wff1_d = P('wff1', [L, 32, 2, 128, 8, 128])
    wff2_d = P('wff2', [L, 4, 8, 128, 8, 128])
    wproj_d = P('wproj', [2, 8, 128, 512])
    out_d = nc.declare_dram_parameter('out', [N, D], f32, isOutput=True)

    R = f32r

    from contextlib import ExitStack
    with ExitStack() as _es:
        _es.enter_context(
            nc.allow_low_precision("bf16 weights/activations; fp32 accumulation"))
        tc = _es.enter_context(tile.TileContext(nc))
        pool = lambda **kw: _es.enter_context(tc.tile_pool(**kw))
        cpool = pool(name="const", bufs=1)
        rpool = pool(name="res", bufs=1)
        wqp = pool(name="wq", bufs=2)
        wsp = pool(name="wsmall", bufs=2)
        wop = pool(name="wo", bufs=9)
        wf1p = pool(name="wff1", bufs=5)
        wf2p = pool(name="wff2", bufs=2)
        wpp = pool(name="wproj", bufs=8)
        scrp = pool(name="scr", bufs=3)
        rotp = pool(name="rot", bufs=2)
        attp = pool(name="attn", bufs=3)
        sgp = pool(name="sg", bufs=3)
        vecp = pool(name="vec", bufs=3)
        ps = pool(name="ps", bufs=4, space="PSUM")
        psacc = pool(name="psacc", bufs=2, space="PSUM")
        psbc = pool(name="psbc", bufs=2, space="PSUM")
        if True:

            # ---------- constants ----------
            cosx_t = cpool.tile([128, NP], bf16, tag="cosx")
            nc.scalar.dma_start(cosx_t[:], cosx_d[:])
            sinx_t = cpool.tile([128, NP], bf16, tag="sinx")
            nc.scalar.dma_start(sinx_t[:], sinx_d[:])
            p2q_t = cpool.tile([128, 128], R, tag="p2q")
            nc.scalar.dma_start(p2q_t[:], p2q_d[:].bitcast(R))
            p2k_t = cpool.tile([64, 64], R, tag="p2k")
            nc.scalar.dma_start(p2k_t[:], p2k_d[:].bitcast(R))
            mh2_t = cpool.tile([128, 2], R, tag="mh2")
            nc.scalar.dma_start(mh2_t[:], mh2_d[:].bitcast(R))
            bch2_t = cpool.tile([2, 128], R, tag="bch2")
            nc.scalar.dma_start(bch2_t[:], bch2_d[:].bitcast(R))
            mh64_t = cpool.tile([64, 1], R, tag="mh64")
            nc.scalar.dma_start(mh64_t[:], mh64_d[:].bitcast(R))
            onesD_t = cpool.tile([128, 1], R, tag="onesD")
            nc.scalar.dma_start(onesD_t[:], onesD_d[:].bitcast(R))
            one1_t = cpool.tile([1, 2], R, tag="one1")
            nc.scalar.dma_start(one1_t[:], one1_d[:].bitcast(R))
            onesc_t = cpool.tile([128, 64], R, tag="onesc")
            nc.scalar.dma_start(onesc_t[:], onesc_d[:].bitcast(R))
            invgo_t = cpool.tile([128, L * 8], f32, tag="invgo")
            nc.scalar.dma_start(invgo_t[:], invgo_d[:])

            epsc = cpool.tile([128, 1], f32, tag="epsc")
            nc.gpsimd.memset(epsc[:], EPS)
            eps12 = cpool.tile([128, 1], f32, tag="eps12")
            nc.gpsimd.memset(eps12[:], 1e-12)

            # ---------- persistent activations ----------
            xt = rpool.tile([128, NMT * NP], R, tag="x")
            for mt in range(NMT):
                nc.gpsimd.dma_start(
                    xt[:, mt * NP:(mt + 1) * NP],
                    xT_d[mt * 128:(mt + 1) * 128, :].bitcast(R))
            o2xn = rpool.tile([128, NMT * NP], bf16, tag="o2xn")
            qhat = rpool.tile([128, 4 * NP], R, tag="qhat")
            khat = rpool.tile([128, NP], R, tag="khat")
            vaug = rpool.tile([128, 5 * 65], bf16, tag="vaug")
            oT = rpool.tile([64, H * NP], bf16, tag="oT")
            sff = rpool.tile([128, NMT * NP], bf16, tag="sff")
            m_sb = rpool.tile([1, NP], R, tag="m_sb")
            s_sb = rpool.tile([1, NP], R, tag="s_sb")
            r_sb = rpool.tile([1, NP], R, tag="r_sb")
            mo_sb = rpool.tile([1, NP], bf16, tag="mo_sb")
            for c in range(5):
                nc.gpsimd.dma_start(vaug[:, c * 65 + 64:c * 65 + 65], onesb_d[:])
            # ebias is large (4.3MB) and first needed by layer-0 attention:
            # issue after xT/layer-0 weight DMAs so they aren't starved.
            eb_t = cpool.tile([128, H * NSLOT * QT], bf16, tag="eb")
            nc.scalar.dma_start(eb_t[:], eb_d[:])

            def rot_l2(pq, np_, p2t, mht, qo, qw, dst):
                """rotary + l2norm(*4). pq: [np_, QT] PSUM -> dst (tile, off)."""
                qs = rotp.tile([128, QT], R, tag="qs")
                nc.scalar.activation(qs[0:np_, :], pq[:, :], AF.Copy)
                rps = ps.tile([128, QT], f32, tag="mm")
                nc.tensor.matmul(rps[0:np_, :], p2t[:], qs[0:np_, :],
                                 start=True, stop=True)
                t1 = rotp.tile([128, QT], R, tag="t1")
                nc.vector.scalar_tensor_tensor(
                    t1[0:np_, :], qs[0:np_, :], 1.0, cosx_t[0:np_, qo:qo + qw],
                    ALU.mult, ALU.mult)
                t2 = rotp.tile([128, QT], R, tag="t2")
                nc.vector.scalar_tensor_tensor(
                    t2[0:np_, :], rps[0:np_, :], 1.0, sinx_t[0:np_, qo:qo + qw],
                    ALU.mult, ALU.mult)
                qr = rotp.tile([128, QT], R, tag="qr")
                nc.gpsimd.tensor_tensor(qr[0:np_, :], t1[0:np_, :], t2[0:np_, :],
                                        ALU.add)
                sq = rotp.tile([128, QT], R, tag="sq")
                nc.scalar.activation(sq[0:np_, :], qr[0:np_, :], AF.Square)
                nh = np_ // 64
                ssq = psbc.tile([1, QT] if nh == 1 else [2, QT], f32, tag="bc")
                nc.tensor.matmul(ssq[:], mht[:], sq[0:np_, :], start=True, stop=True)
                sh = vecp.tile([2, QT], f32, tag="sh")
                nc.scalar.activation(sh[0:nh, :], ssq[:], AF.Sqrt,
                                     scale=1.0 / 16.0, bias=eps12[0:nh, :])
                rh = vecp.tile([2, QT], bf16, tag="rh")
                nc.vector.reciprocal(rh[0:nh, :], sh[0:nh, :])
                dt_, off = dst
                if nh == 1:
                    bcq = rotp.tile([128, QT], R, tag="bcq")
                    nc.gpsimd.partition_broadcast(bcq[0:64, :].bitcast(f32), rh[0:1, :].bitcast(f32), 64)
                    nc.vector.scalar_tensor_tensor(
                        dt_[0:np_, off:off + qw], qr[0:np_, :], 1.0,
                        bcq[0:np_, :], ALU.mult, ALU.mult)
                else:
                    bcp = psbc.tile([128, QT], f32, tag="bc")
                    nc.tensor.matmul(bcp[:], bch2_t[:], rh[0:nh, :],
                                     start=True, stop=True)
                    nc.vector.scalar_tensor_tensor(
                        dt_[0:np_, off:off + qw], qr[0:np_, :], 1.0,
                        bcp[0:np_, :], ALU.mult, ALU.mult)

            # ================= layers =================
            for l in range(L):
                # ---- per-layer small weights ----
                wkt = wsp.tile([128, 8 * DH], R, tag="wk")
                nc.gpsimd.dma_start(
                    wkt[:].rearrange("p (c m) -> p c m", c=8), wk_d[l].bitcast(R))
                wvt = wsp.tile([128, 8 * DH], R, tag="wv")
                nc.gpsimd.dma_start(
                    wvt[:].rearrange("p (c m) -> p c m", c=8), wv_d[l].bitcast(R))
                kvec_t = wsp.tile([1, 4 * DH], R, tag="kvec")
                nc.gpsimd.dma_start(kvec_t[:], kvec_d[l].bitcast(R))
                wk1n_t = kvec_t[0:1, 0:DH]
                bkp_t = kvec_t[0:1, DH:2 * DH]
                wv1n_t = kvec_t[0:1, 2 * DH:3 * DH]
                bvv_t = kvec_t[0:1, 3 * DH:4 * DH]
                wq1n_t = wsp.tile([1, 4 * 128], R, tag="wq1n")
                nc.gpsimd.dma_start(wq1n_t[:], wq1n_d[l].bitcast(R))
                wo1n_t = wsp.tile([64, 8], bf16, tag="wo1n")
                nc.gpsimd.dma_start(wo1n_t[:], wo1n_d[l])
                gorow_t = wsp.tile([1, D], bf16, tag="gorow", bufs=1)
                nc.gpsimd.dma_start(gorow_t[:], gorow_d[0:1, l * D:(l + 1) * D])

                # ---- A: attn stats (m, s, r per token) ----
                for (qo, qw) in QTS:
                    s1 = psbc.tile([1, QT], f32, tag="bc")
                    s2 = psbc.tile([1, QT], f32, tag="bc")
                    for mt in range(NMT):
                        seg = xt[:, mt * NP + qo:mt * NP + qo + qw]
                        sq = scrp.tile([128, QT], R, tag="sq")
                        nc.scalar.activation(sq[:], seg, AF.Square)
                        nc.tensor.matmul(s1[:], onesD_t[:], seg,
                                         start=(mt == 0), stop=(mt == NMT - 1))
                        nc.tensor.matmul(s2[:], onesD_t[:], sq[:],
                                         start=(mt == 0), stop=(mt == NMT - 1))
                    nc.vector.tensor_copy(m_sb[0:1, qo:qo + qw], s1[:])
                    msq = vecp.tile([1, QT], f32, tag="msq")
                    nc.scalar.activation(msq[:], s1[:], AF.Square)
                    v_v = vecp.tile([1, QT], f32, tag="v")
                    nc.vector.scalar_tensor_tensor(v_v[:], s2[:], 1.0, msq[:],
                                                   ALU.mult, ALU.subtract)
                    nc.scalar.activation(s_sb[0:1, qo:qo + qw], v_v[:], AF.Sqrt,
                                         bias=epsc[0:1, :])
                    nc.vector.reciprocal(r_sb[0:1, qo:qo + qw],
                                         s_sb[0:1, qo:qo + qw])

                # ---- B/C/D: K, Q, V -- mains pipelined ahead of the
                # stats-dependent finishers (corrections + rotary/l2norm) ----
                def k_main(qi):
                    qo, qw = QTS[qi]
                    pk = ps.tile([64, QT], f32, tag="mm")
                    for c in range(8):
                        nc.tensor.matmul(pk[:], wkt[:, c * DH:(c + 1) * DH],
                                         xt[:, c * NP + qo:c * NP + qo + qw],
                                         start=(c == 0), stop=False)
                    return pk

                def k_fin(qi, pk):
                    qo, qw = QTS[qi]
                    nc.tensor.matmul(pk[:], wk1n_t, m_sb[0:1, qo:qo + qw],
                                     start=False, stop=False)
                    nc.tensor.matmul(pk[:], bkp_t, s_sb[0:1, qo:qo + qw],
                                     start=False, stop=True)
                    rot_l2(pk, 64, p2k_t, mh64_t, qo, qw, (khat, qo))

                def q_main(p, wqt, qi):
                    qo, qw = QTS[qi]
                    pq = ps.tile([128, QT], f32, tag="mm")
                    for c in range(8):
                        nc.tensor.matmul(pq[:], wqt[:, c * 128:(c + 1) * 128],
                                         xt[:, c * NP + qo:c * NP + qo + qw],
                                         start=(c == 0), stop=False)
                    return pq

                def q_fin(p, qi, pq):
                    qo, qw = QTS[qi]
                    nc.tensor.matmul(pq[:], wq1n_t[0:1, p * 128:(p + 1) * 128],
                                     m_sb[0:1, qo:qo + qw],
                                     start=False, stop=True)
                    rot_l2(pq, 128, p2q_t, mh2_t, qo, qw, (qhat, p * NP + qo))

                def v_chunk(t):
                    to, tw = TCH[t]
                    rtk = psbc.tile([128, 2], f32, tag="bc")
                    nc.tensor.matmul(rtk[0:tw, :], r_sb[0:1, to:to + tw],
                                     one1_t[:], start=True, stop=True)
                    pv = ps.tile([128, DH], f32, tag="mm")
                    for c in range(8):
                        nc.tensor.matmul(pv[0:tw, :],
                                         xt[:, c * NP + to:c * NP + to + tw],
                                         wvt[:, c * DH:(c + 1) * DH],
                                         start=(c == 0), stop=False)
                    nc.tensor.matmul(pv[0:tw, :], m_sb[0:1, to:to + tw],
                                     wv1n_t, start=False, stop=False)
                    nc.tensor.matmul(pv[0:tw, :], s_sb[0:1, to:to + tw],
                                     bvv_t, start=False, stop=True)
                    with tc.high_priority():
                        rts = vecp.tile([128, 1], f32, tag="rts")
                        nc.vector.tensor_copy(rts[0:tw, :], rtk[0:tw, 0:1])
                        nc.scalar.activation(vaug[0:tw, t * 65:t * 65 + DH],
                                             pv[0:tw, :], AF.Copy,
                                             scale=rts[0:tw, :])

                wqts = {}
                def load_wq(p):
                    wqt = wqp.tile([128, 8 * 128], R, tag="wq")
                    nc.sync.dma_start(
                        wqt[:].rearrange("p (c m) -> p c m", c=8),
                        wq_d[l, p].bitcast(R))
                    wqts[p] = wqt

                load_wq(0)
                pk0 = k_main(0)
                pk1 = k_main(1)
                load_wq(1)
                pq = {}
                pq[(0, 0)] = q_main(0, wqts[0], 0)
                k_fin(0, pk0)
                pq[(0, 1)] = q_main(0, wqts[0], 1)
                k_fin(1, pk1)
                nc.gpsimd.dma_start(khat[64:128, 0:NP], khat[0:64, 0:NP])
                nc.gpsimd.dma_start(khat[:, NK:NK + 1], nullk_d[l])
                load_wq(2)
                pq[(1, 0)] = q_main(1, wqts[1], 0)
                q_fin(0, 0, pq[(0, 0)])
                pq[(1, 1)] = q_main(1, wqts[1], 1)
                q_fin(0, 1, pq[(0, 1)])
                load_wq(3)
                pq[(2, 0)] = q_main(2, wqts[2], 0)
                q_fin(1, 0, pq[(1, 0)])
                pq[(2, 1)] = q_main(2, wqts[2], 1)
                q_fin(1, 1, pq[(1, 1)])
                pq[(3, 0)] = q_main(3, wqts[3], 0)
                q_fin(2, 0, pq[(2, 0)])
                pq[(3, 1)] = q_main(3, wqts[3], 1)
                q_fin(2, 1, pq[(2, 1)])
                v_chunk(0)
                v_chunk(1)
                v_chunk(2)
                q_fin(3, 0, pq[(3, 0)])
                v_chunk(3)
                v_chunk(4)
                q_fin(3, 1, pq[(3, 1)])
                # null v at key row NK=515 -> chunk 4, partition 3
                nc.gpsimd.dma_start(vaug[3:4, 4 * 65:4 * 65 + DH], nullv_d[l])

                # ---- E: attention (unnormalized softmax) ----
                for h in range(H):
                    hb = 64 * (h & 1)
                    qcol = (h >> 1) * NP
                    for qi, (qo, qw) in enumerate(QTS):
                        av = psacc.tile([128, QT], f32, tag="av")
                        chunks = CHUNKS[qi]
                        for ci, c in enumerate(chunks):
                            kw = KW[c]
                            slot = (ci if qi == 0 else 4 + ci)
                            sp = ps.tile([128, QT], f32, tag="mm")
                            nc.tensor.matmul(
                                sp[0:kw, :],
                                khat[hb:hb + 64, 128 * c:128 * c + kw],
                                qhat[hb:hb + 64, qcol + qo:qcol + qo + qw],
                                start=True, stop=True)
                            aex = attp.tile([128, QT], bf16, tag="aex")
                            nc.scalar.activation(aex[0:kw, :], sp[0:kw, :], AF.Exp)
                            au = attp.tile([128, QT], bf16, tag="au")
                            ebs = (h * NSLOT + slot) * QT
                            with tc.high_priority():
                                nc.vector.scalar_tensor_tensor(
                                    au[0:kw, :], aex[0:kw, :], 1.0,
                                    eb_t[0:kw, ebs:ebs + QT], ALU.mult, ALU.mult)
                            nc.tensor.matmul(av[0:65, :],
                                             vaug[0:kw, c * 65:(c + 1) * 65],
                                             au[0:kw, :], start=(ci == 0),
                                             stop=(ci == len(chunks) - 1))
                        with tc.high_priority():
                            rd = scrp.tile([128, QT], R, tag="rd", bufs=2)
                            nc.vector.reciprocal(rd[64:65, :], av[64:65, :])
                            bco = psbc.tile([64, QT], f32, tag="bc")
                            nc.tensor.matmul(bco[0:64, :], onesc_t[64:65, :],
                                             rd[64:65, :], start=True, stop=True)
                            bcos = attp.tile([64, QT], bf16, tag="bcos", bufs=2)
                            nc.vector.tensor_copy(bcos[:], bco[0:64, :])
                            nc.vector.scalar_tensor_tensor(
                                oT[0:64, h * NP + qo:h * NP + qo + qw],
                                av[0:64, :], 1.0, bcos[:], ALU.mult, ALU.mult)

                # ---- F: Wo + out-LN + residual ----
                wots = []
                for mt in range(NMT):
                    wot = wop.tile([64, 8 * 128], bf16, tag="wo")
                    nc.gpsimd.dma_start(
                        wot[:].rearrange("p (c m) -> p c m", c=8), wo_d[l, mt])
                    wots.append(wot)
                for (qo, qw) in QTS:
                    mo = psbc.tile([1, QT], f32, tag="bc")
                    for c in range(8):
                        nc.tensor.matmul(mo[:], wo1n_t[:, c:c + 1],
                                         oT[0:64, c * NP + qo:c * NP + qo + qw],
                                         start=(c == 0), stop=(c == 7))
                    with tc.high_priority():
                        nc.vector.tensor_copy(mo_sb[0:1, qo:qo + qw], mo[:])
                    s2c = psbc.tile([1, QT], f32, tag="bc")
                    for mt in range(NMT):
                        pl = ps.tile([128, QT], f32, tag="mm")
                        for c in range(8):
                            nc.tensor.matmul(pl[:],
                                             wots[mt][:, c * 128:(c + 1) * 128],
                                             oT[0:64, c * NP + qo:c * NP + qo + qw],
                                             start=(c == 0), stop=False)
                        nc.tensor.matmul(pl[:],
                                         gorow_t[0:1, mt * 128:(mt + 1) * 128],
                                         mo_sb[0:1, qo:qo + qw],
                                         start=False, stop=True)
                        sqo = scrp.tile([128, QT], R, tag="sq")
                        nc.scalar.activation(sqo[:], pl[:], AF.Square,
                                             scale=invgo_t[:, l * 8 + mt:
                                                           l * 8 + mt + 1])
                        nc.tensor.matmul(s2c[:], onesD_t[:], sqo[:],
                                         start=(mt == 0), stop=(mt == NMT - 1))
                        nc.scalar.activation(
                            o2xn[:, mt * NP + qo:mt * NP + qo + qw],
                            pl[:], AF.Copy)
                    so = vecp.tile([1, QT], f32, tag="so")
                    nc.scalar.activation(so[:], s2c[:], AF.Sqrt, bias=epsc[0:1, :])
                    ro = vecp.tile([1, QT], bf16, tag="ro")
                    nc.vector.reciprocal(ro[:], so[:])
                    rb = sgp.tile([128, QT], f32, tag="rbb", bufs=2)
                    nc.gpsimd.partition_broadcast(rb[:], ro[:], 128)
                    for mt in range(NMT):
                        eng = nc.vector if mt % 2 == 0 else nc.gpsimd
                        tt = scrp.tile([128, QT], bf16, tag="tt")
                        eng.tensor_tensor(
                            tt[:], o2xn[:, mt * NP + qo:mt * NP + qo + qw],
                            rb[:], ALU.mult)
                        xcols = xt[:, mt * NP + qo:mt * NP + qo + qw]
                        eng.tensor_tensor(xcols, xcols, tt[:], ALU.add)

                # ---- G: ff-LN -> xn (per qtile, interleaved with FFN
                # block 0 so PE keeps working during the q1 stats chain) ----
                def ff_ln(qi):
                    qo, qw = QTS[qi]
                    s1 = psbc.tile([1, QT], f32, tag="bc")
                    s2 = psbc.tile([1, QT], f32, tag="bc")
                    for mt in range(NMT):
                        seg = xt[:, mt * NP + qo:mt * NP + qo + qw]
                        sq = scrp.tile([128, QT], R, tag="sq")
                        nc.scalar.activation(sq[:], seg, AF.Square)
                        nc.tensor.matmul(s1[:], onesD_t[:], seg,
                                         start=(mt == 0), stop=(mt == NMT - 1))
                        nc.tensor.matmul(s2[:], onesD_t[:], sq[:],
                                         start=(mt == 0), stop=(mt == NMT - 1))
                    msq = vecp.tile([1, QT], f32, tag="vf")
                    nc.scalar.activation(msq[:], s1[:], AF.Square)
                    v_v = vecp.tile([1, QT], f32, tag="vf")
                    nc.vector.scalar_tensor_tensor(v_v[:], s2[:], 1.0, msq[:],
                                                   ALU.mult, ALU.subtract)
                    sf = vecp.tile([1, QT], f32, tag="vf")
                    nc.scalar.activation(sf[:], v_v[:], AF.Sqrt, bias=epsc[0:1, :])
                    rf = vecp.tile([1, QT], f32, tag="vbr")
                    nc.vector.reciprocal(rf[:], sf[:])
                    mrv = vecp.tile([1, QT], f32, tag="vbr")
                    nc.vector.scalar_tensor_tensor(mrv[:], s1[:], 1.0, rf[:],
                                                   ALU.mult, ALU.mult)
                    rb = sgp.tile([128, QT], f32, tag="rbb", bufs=2)
                    nc.gpsimd.partition_broadcast(rb[:], rf[:], 128)
                    mrb = sgp.tile([128, QT], f32, tag="mrbb", bufs=2)
                    nc.gpsimd.partition_broadcast(mrb[:], mrv[:], 128)
                    for mt in range(NMT):
                        eng = nc.vector if mt % 2 == 0 else nc.gpsimd
                        tt = scrp.tile([128, QT], bf16, tag="tt")
                        eng.tensor_tensor(
                            tt[:], xt[:, mt * NP + qo:mt * NP + qo + qw],
                            rb[:], ALU.mult)
                        eng.tensor_tensor(
                            o2xn[:, mt * NP + qo:mt * NP + qo + qw],
                            tt[:], mrb[:], ALU.subtract)

                # ---- H: FFN (SwiGLU), 4 halves of 8 s-blocks ----
                def ffn1_qt(mi, wga, wgg, qi):
                    qo, qw = QTS[qi]
                    pg = ps.tile([128, QT], f32, tag="mm")
                    for c in range(8):
                        nc.tensor.matmul(pg[:], wgg[:, c * 128:(c + 1) * 128],
                                         o2xn[:, c * NP + qo:c * NP + qo + qw],
                                         start=(c == 0), stop=(c == 7))
                    sig = sgp.tile([128, QT], bf16, tag="sig")
                    nc.scalar.activation(sig[:], pg[:], AF.Sigmoid)
                    pa = ps.tile([128, QT], f32, tag="mm")
                    for c in range(8):
                        nc.tensor.matmul(pa[:], wga[:, c * 128:(c + 1) * 128],
                                         o2xn[:, c * NP + qo:c * NP + qo + qw],
                                         start=(c == 0), stop=(c == 7))
                    gs = sgp.tile([128, QT], bf16, tag="ag")
                    nc.vector.scalar_tensor_tensor(
                        gs[:], pg[:], 1.0, sig[:], ALU.mult, ALU.mult)
                    nc.vector.scalar_tensor_tensor(
                        sff[:, mi * NP + qo:mi * NP + qo + qw],
                        pa[:], 1.0, gs[:], ALU.mult, ALU.mult)

                def load_wff1(m):
                    wga = wf1p.tile([128, 8 * 128], bf16, tag="wff1")
                    nc.sync.dma_start(
                        wga[:].rearrange("p (c m) -> p c m", c=8),
                        wff1_d[l, m, 0])
                    wgg = wf1p.tile([128, 8 * 128], bf16, tag="wff1")
                    nc.sync.dma_start(
                        wgg[:].rearrange("p (c m) -> p c m", c=8),
                        wff1_d[l, m, 1])
                    return wga, wgg

                ff_ln(0)
                w00 = load_wff1(0)
                ffn1_qt(0, w00[0], w00[1], 0)
                ff_ln(1)
                ffn1_qt(0, w00[0], w00[1], 1)
                for half in range(4):
                    for mi in range(8):
                        if half == 0 and mi == 0:
                            continue
                        m = half * 8 + mi
                        wga, wgg = load_wff1(m)
                        for qi in range(2):
                            ffn1_qt(mi, wga, wgg, qi)
                    for mt in range(NMT):
                        w2 = wf2p.tile([128, 8 * 128], bf16, tag="wff2")
                        nc.sync.dma_start(
                            w2[:].rearrange("p (c m) -> p c m", c=8),
                            wff2_d[l, half, mt])
                        for (qo, qw) in QTS:
                            pl = ps.tile([128, QT], f32, tag="mm")
                            for c in range(8):
                                nc.tensor.matmul(
                                    pl[:], w2[:, c * 128:(c + 1) * 128],
                                    sff[:, c * NP + qo:c * NP + qo + qw],
                                    start=(c == 0), stop=(c == 7))
                            xcols = xt[:, mt * NP + qo:mt * NP + qo + qw]
                            nc.gpsimd.tensor_tensor(xcols, xcols, pl[:], ALU.add)

            # ================= final stable LN + Wproj =================
            xm = cpool.tile([128, NP], bf16, tag="xm")
            nc.vector.tensor_tensor(xm[:], xt[:, 0:NP], xt[:, NP:2 * NP], ALU.max)
            for mt in range(2, NMT):
                nc.vector.tensor_tensor(xm[:], xm[:], xt[:, mt * NP:(mt + 1) * NP],
                                        ALU.max)
            mxb = cpool.tile([128, NP], bf16, tag="mxb")
            from concourse import bass_isa
            nc.gpsimd.partition_all_reduce(mxb[:], xm[:], 128, bass_isa.ReduceOp.max)

            for (qo, qw) in QTS:
                s1 = psbc.tile([1, QT], f32, tag="bc")
                s2 = psbc.tile([1, QT], f32, tag="bc")
                for mt in range(NMT):
                    seg = xt[:, mt * NP + qo:mt * NP + qo + qw]
                    sq = scrp.tile([128, QT], R, tag="sq")
                    nc.scalar.activation(sq[:], seg, AF.Square)
                    nc.tensor.matmul(s1[:], onesD_t[:], seg,
                                     start=(mt == 0), stop=(mt == NMT - 1))
                    nc.tensor.matmul(s2[:], onesD_t[:], sq[:],
                                     start=(mt == 0), stop=(mt == NMT - 1))
                msq = vecp.tile([1, QT], f32, tag="msq")
                nc.scalar.activation(msq[:], s1[:], AF.Square)
                v_v = vecp.tile([1, QT], f32, tag="v")
                nc.vector.scalar_tensor_tensor(v_v[:], s2[:], 1.0, msq[:],
                                               ALU.mult, ALU.subtract)
                mxsq = vecp.tile([1, QT], f32, tag="mxsq")
                nc.scalar.activation(mxsq[:], mxb[0:1, qo:qo + qw], AF.Square)
                veps = vecp.tile([1, QT], f32, tag="veps")
                nc.vector.scalar_tensor_tensor(veps[:], mxsq[:], EPS, v_v[:],
                                               ALU.mult, ALU.add)
                sf = vecp.tile([1, QT], f32, tag="sf")
                nc.scalar.activation(sf[:], veps[:], AF.Sqrt)
                rf = vecp.tile([1, QT], bf16, tag="rf")
                nc.vector.reciprocal(rf[:], sf[:])
                mrv = vecp.tile([1, QT], bf16, tag="mrv")
                nc.vector.scalar_tensor_tensor(mrv[:], s1[:], 1.0, rf[:],
                                               ALU.mult, ALU.mult)
                rb = sgp.tile([128, QT], bf16, tag="rbb")
                nc.gpsimd.partition_broadcast(rb[:], rf[:], 128)
                mrb = sgp.tile([128, QT], bf16, tag="mrbb")
                nc.gpsimd.partition_broadcast(mrb[:], mrv[:], 128)
                for mt in range(NMT):
                    tt = scrp.tile([128, QT], bf16, tag="tt")
                    nc.vector.scalar_tensor_tensor(
                        tt[:], xt[:, mt * NP + qo:mt * NP + qo + qw], 1.0,
                        rb[:], ALU.mult, ALU.mult)
                    nc.vector.scalar_tensor_tensor(
                        o2xn[:, mt * NP + qo:mt * NP + qo + qw],
                        tt[:], 1.0, mrb[:], ALU.mult, ALU.subtract)

            for half in range(2):
                wps = []
                for c in range(8):
                    wp = wpp.tile([128, 512], bf16, tag=f"wp{c}")
                    nc.sync.dma_start(wp[:], wproj_d[half, c])
                    wps.append(wp)
                for t, (to, tw) in enumerate(TCH):
                    rtw = min(tw, max(0, N - to))
                    if rtw == 0:
                        continue
                    pn = psacc.tile([128, 512], f32, tag="av")
                    for c in range(8):
                        nc.tensor.matmul(pn[0:tw, :],
                                         o2xn[:, c * NP + to:c * NP + to + tw],
                                         wps[c][:], start=(c == 0), stop=(c == 7))
                    st = scrp.tile([128, 512], f32, tag="outst", bufs=2)
                    nc.scalar.activation(st[0:rtw, :], pn[0:rtw, :], AF.Copy)
                    nc.sync.dma_start(out_d[to:to + rtw, half * 512:(half + 1) * 512],
                                      st[0:rtw, :])

    nc.compile()
    return nc


_CACHE = {}


def _get_program():
    if 'nc' not in _CACHE:
        _CACHE['nc'] = _build()
    return _CACHE['nc']


def kernel(**inputs) -> np.ndarray:
    from concourse.bass_utils import run_bass_kernel_spmd
    host = _host_prep(inputs)
    nc = _get_program()
    shared = {k: v for k, v in host.items() if k != 'xT'}
    in_maps = [dict(shared, xT=np.ascontiguousarray(host['xT'][b])) for b in range(B)]
    res = run_bass_kernel_spmd(nc, in_maps, list(range(B)))
    out = np.stack([res.results[b]['out'] for b in range(B)], axis=0)
    _CACHE['last_results'] = res
    return out
